# revision 7
# baseline (speedup 1.0000x reference)
"""Bass/Trainium2 kernel for nn_DirectedLayer (GNN message passing).

Computes, for a directed graph with E edges and N nodes:
    head, tail = split(efeat, 2, axis=-1)
    mean_in  = segment_mean(head, dst, N)
    mean_out = segment_mean(tail, src, N)
    nfeat = 0.5 * (mean_in + mean_out)
    out = concat([concat([nfeat[src], head]) @ W_head + b_head,
                  concat([nfeat[dst], tail]) @ W_tail + b_tail], axis=-1)

Distribution over 8 NeuronCores:
  Phase A: nodes sharded by contiguous windows (<=128 nodes each); each core
           computes the scaled segment sums for its windows via one-hot
           matmuls over host-sorted edge payloads, then projects them through
           the first half of each weight matrix (bias folded in).
  Phase B: AllGather of the two projected node tables.
  Phase C: edges sharded contiguously; each core streams its transposed edge
           features, multiplies by the second half of the weights, gathers the
           projected node rows with 2-pass int16 dma_gather, adds, and stores.
"""

import sys

for p in ("/opt/trn_rl_repo/concourse", "/opt/trn_rl_repo"):
    if p not in sys.path:
        sys.path.insert(0, p)

import numpy as np

import concourse.bass as bass
import concourse.bacc as bacc
import concourse.mybir as mybir
import concourse.tile as tile
from concourse.bass_utils import run_bass_kernel_spmd

N_CORES = 8
D = 128          # feature dim per half
T = 9            # edge tiles (of 128) per window, per direction
CH = 2048        # phase-C chunk size in edges
F32 = mybir.dt.float32
I16 = mybir.dt.int16


# ---------------------------------------------------------------- host prep

def _window_partition(cin, cout, N):
    """Greedy split of nodes 0..N-1 into consecutive windows with
    <=128 nodes (127 for window 0; slot 0 is a reserved zero row) and
    <=T*128 incident edges per direction."""
    cap_e = T * 128
    w_id = np.empty(N, dtype=np.int64)
    slot = np.empty(N, dtype=np.int64)
    w = 0
    nn = 0
    de = 0
    se = 0
    first_slot = 1  # window 0 reserves slot 0
    for n in range(N):
        cap_n = 128 - (1 if w == 0 else 0)
        if nn > 0 and (
            nn >= cap_n or de + cin[n] > cap_e or se + cout[n] > cap_e
        ):
            w += 1
            nn = 0
            de = 0
            se = 0
            first_slot = 0
        w_id[n] = w
        slot[n] = nn + (first_slot if w == 0 else 0)
        nn += 1
        de += cin[n]
        se += cout[n]
    return w_id, slot, w + 1


def _dir_payload(efeat_half, nidx, wgt_node, w_id, slot, core, W_max, order):
    """Payload/one-hot metadata for one direction on one core.

    nidx: per-edge node index (dst or src); order: edges sorted by w_id[nidx].
    Returns (pay [W_max*T*128, D], loff [128, W_max*T], wcol [128, W_max*T]).
    """
    w_of_e = w_id[nidx[order]]
    lo_w = core * W_max
    hi_w = (core + 1) * W_max
    m = (w_of_e >= lo_w) & (w_of_e < hi_w)
    ed = order[m]
    we = w_of_e[m] - lo_w
    # rank of each edge within its window (edges are grouped by window)
    starts = np.searchsorted(we, np.arange(W_max))
    rank = np.arange(len(ed)) - starts[we]
    pos = we * (T * 128) + rank
    assert rank.max(initial=0) < T * 128

    rows = W_max * T * 128
    pay = np.zeros((rows, D), dtype=np.float32)
    loff = np.zeros(rows, dtype=np.float32)
    wcol = np.zeros(rows, dtype=np.float32)
    pay[pos] = efeat_half[ed]
    loff[pos] = slot[nidx[ed]]
    wcol[pos] = wgt_node[nidx[ed]]
    return (
        pay,
        np.ascontiguousarray(loff.reshape(-1, 128).T),
        np.ascontiguousarray(wcol.reshape(-1, 128).T),
    )


def _wrap_idx(gidx, chunks):
    """Pack per-chunk wrapped int16 index table [128, ceil(E_c/16)]."""
    cols = sum(ch for _, ch in chunks) // 16
    out = np.empty((16, cols), dtype=np.int16)
    c0 = 0
    for e0, ch in chunks:
        out[:, c0 : c0 + ch // 16] = (
            gidx[e0 : e0 + ch].reshape(ch // 16, 16).T
        )
        c0 += ch // 16
    return np.ascontiguousarray(np.tile(out, (8, 1)))


def _chunks(E_c):
    out = []
    e0 = 0
    while e0 < E_c:
        ch = min(CH, E_c - e0)
        if ch % 16:
            # keep every chunk a multiple of 16 except never needed for our E
            raise ValueError("edge shard must be a multiple of 16")
        out.append((e0, ch))
        e0 += ch
    return out


def _prep(efeat, src, dst, N, W_head, b_head, W_tail, b_tail):
    E = src.shape[0]
    assert E % N_CORES == 0
    E_c = E // N_CORES
    src = src.astype(np.int64)
    dst = dst.astype(np.int64)

    cin = np.bincount(dst, minlength=N)
    cout = np.bincount(src, minlength=N)
    w_in = (0.5 / np.maximum(cin, 1)).astype(np.float32)
    w_out = (0.5 / np.maximum(cout, 1)).astype(np.float32)

    w_id, slot, W_total = _window_partition(cin, cout, N)
    W_max = -(-W_total // N_CORES)
    # row R-1 must stay a zero row (junk target of the hi gather pass)
    if W_total == N_CORES * W_max:
        last = np.where(w_id == W_total - 1)[0]
        if slot[last].max() >= 127:
            W_max += 1
    R = N_CORES * W_max * 128
    assert R <= 65536, f"node table too large for 2-pass int16 gather: {R}"
    two_pass = R > 32768
    hibase = R - 32768

    row = w_id * 128 + slot  # global row of each node in the gathered tables

    order_d = np.argsort(w_id[dst], kind="stable")
    order_s = np.argsort(w_id[src], kind="stable")

    gh = row[src]  # phase-C gather rows for the head output (nfeat[src])
    gt = row[dst]
    chunks = _chunks(E_c)

    iota = np.tile(np.arange(128, dtype=np.float32), (128, 1))

    in_maps = []
    for c in range(N_CORES):
        pay_d, loff_d, w_d = _dir_payload(
            efeat[:, :D], dst, w_in, w_id, slot, c, W_max, order_d
        )
        pay_s, loff_s, w_s = _dir_payload(
            efeat[:, D:], src, w_out, w_id, slot, c, W_max, order_s
        )
        bias1 = np.zeros((1, W_max * 128), dtype=np.float32)
        mine = (w_id >= c * W_max) & (w_id < (c + 1) * W_max)
        bias1[0, (w_id[mine] - c * W_max) * 128 + slot[mine]] = 1.0

        sl = slice(c * E_c, (c + 1) * E_c)
        ghc = gh[sl]
        gtc = gt[sl]
        if two_pass:
            glo_h = np.where(ghc < 32768, ghc, 0).astype(np.int16)
            ghi_h = np.where(ghc >= 32768, ghc - hibase, 32767).astype(np.int16)
            glo_t = np.where(gtc < 32768, gtc, 0).astype(np.int16)
            ghi_t = np.where(gtc >= 32768, gtc - hibase, 32767).astype(np.int16)
        else:
            glo_h = ghc.astype(np.int16)
            ghi_h = glo_t = ghi_t = None
            glo_t = gtc.astype(np.int16)

        m = {
            "pay_d": pay_d,
            "pay_s": pay_s,
            "loff_d": loff_d,
            "loff_s": loff_s,
            "w_d": w_d,
            "w_s": w_s,
            "bias1": bias1,
            "iota": iota,
            "Wh1": np.ascontiguousarray(W_head[:D]),
            "Wh2": np.ascontiguousarray(W_head[D:]),
            "Wt1": np.ascontiguousarray(W_tail[:D]),
            "Wt2": np.ascontiguousarray(W_tail[D:]),
            "bh": b_head.reshape(1, D).astype(np.float32),
            "bt": b_tail.reshape(1, D).astype(np.float32),
            "efT": np.ascontiguousarray(efeat[sl].T),
            "ixlo_h": _wrap_idx(glo_h, chunks),
            "ixlo_t": _wrap_idx(glo_t, chunks),
        }
        if two_pass:
            m["ixhi_h"] = _wrap_idx(ghi_h, chunks)
            m["ixhi_t"] = _wrap_idx(ghi_t, chunks)
        in_maps.append(m)

    meta = {
        "E_c": E_c,
        "W_max": W_max,
        "R": R,
        "two_pass": two_pass,
        "hibase": hibase,
        "chunks": chunks,
        "idx_cols": sum(ch for _, ch in chunks) // 16,
    }
    return in_maps, meta


# ------------------------------------------------------------- device build

def _build(meta):
    E_c = meta["E_c"]
    W_max = meta["W_max"]
    R = meta["R"]
    two_pass = meta["two_pass"]
    hibase = meta["hibase"]
    chunks = meta["chunks"]
    icols = meta["idx_cols"]
    WT = W_max * T

    nc = bacc.Bacc(None, num_devices=N_CORES, debug=False)

    pay_d = nc.dram_tensor("pay_d", [WT * 128, D], F32, kind="ExternalInput")
    pay_s = nc.dram_tensor("pay_s", [WT * 128, D], F32, kind="ExternalInput")
    loff_d = nc.dram_tensor("loff_d", [128, WT], F32, kind="ExternalInput")
    loff_s = nc.dram_tensor("loff_s", [128, WT], F32, kind="ExternalInput")
    w_d = nc.dram_tensor("w_d", [128, WT], F32, kind="ExternalInput")
    w_s = nc.dram_tensor("w_s", [128, WT], F32, kind="ExternalInput")
    bias1 = nc.dram_tensor("bias1", [1, W_max * 128], F32, kind="ExternalInput")
    iota_in = nc.dram_tensor("iota", [128, 128], F32, kind="ExternalInput")
    Wh1 = nc.dram_tensor("Wh1", [D, D], F32, kind="ExternalInput")
    Wh2 = nc.dram_tensor("Wh2", [D, D], F32, kind="ExternalInput")
    Wt1 = nc.dram_tensor("Wt1", [D, D], F32, kind="ExternalInput")
    Wt2 = nc.dram_tensor("Wt2", [D, D], F32, kind="ExternalInput")
    bh = nc.dram_tensor("bh", [1, D], F32, kind="ExternalInput")
    bt = nc.dram_tensor("bt", [1, D], F32, kind="ExternalInput")
    efT = nc.dram_tensor("efT", [2 * D, E_c], F32, kind="ExternalInput")
    ixlo_h = nc.dram_tensor("ixlo_h", [128, icols], I16, kind="ExternalInput")
    ixlo_t = nc.dram_tensor("ixlo_t", [128, icols], I16, kind="ExternalInput")
    if two_pass:
        ixhi_h = nc.dram_tensor("ixhi_h", [128, icols], I16, kind="ExternalInput")
        ixhi_t = nc.dram_tensor("ixhi_t", [128, icols], I16, kind="ExternalInput")
    out = nc.dram_tensor("out", [E_c, 2 * D], F32, kind="ExternalOutput")

    nph_loc = nc.dram_tensor("nph_loc", [W_max * 128, D], F32)
    npt_loc = nc.dram_tensor("npt_loc", [W_max * 128, D], F32)
    nph_gl = nc.dram_tensor("nph_gl", [R, D], F32, addr_space="Shared")
    npt_gl = nc.dram_tensor("npt_gl", [R, D], F32, addr_space="Shared")

    rg = [list(range(N_CORES))]

    with tile.TileContext(nc) as tc:
        with (
            tc.tile_pool(name="const", bufs=1) as cpool,
            tc.tile_pool(name="sbuf", bufs=2) as pool,
            tc.tile_pool(name="psum", bufs=2, space="PSUM") as pp,
        ):
            iota_t = cpool.tile([128, 128], F32, tag="iota")
            nc.sync.dma_start(out=iota_t[:], in_=iota_in[:])
            wh1_t = cpool.tile([D, D], F32, tag="wh1")
            nc.sync.dma_start(out=wh1_t[:], in_=Wh1[:])
            wh2_t = cpool.tile([D, D], F32, tag="wh2")
            nc.sync.dma_start(out=wh2_t[:], in_=Wh2[:])
            wt1_t = cpool.tile([D, D], F32, tag="wt1")
            nc.sync.dma_start(out=wt1_t[:], in_=Wt1[:])
            wt2_t = cpool.tile([D, D], F32, tag="wt2")
            nc.sync.dma_start(out=wt2_t[:], in_=Wt2[:])
            bh_t = cpool.tile([1, D], F32, tag="bh")
            nc.sync.dma_start(out=bh_t[:], in_=bh[:])
            bt_t = cpool.tile([1, D], F32, tag="bt")
            nc.sync.dma_start(out=bt_t[:], in_=bt[:])
            b1_t = cpool.tile([1, W_max * 128], F32, tag="b1")
            nc.sync.dma_start(out=b1_t[:], in_=bias1[:])
            lo_d_t = cpool.tile([128, WT], F32, tag="lod")
            nc.sync.dma_start(out=lo_d_t[:], in_=loff_d[:])
            lo_s_t = cpool.tile([128, WT], F32, tag="los")
            nc.sync.dma_start(out=lo_s_t[:], in_=loff_s[:])
            wd_t = cpool.tile([128, WT], F32, tag="wd")
            nc.sync.dma_start(out=wd_t[:], in_=w_d[:])
            ws_t = cpool.tile([128, WT], F32, tag="ws")
            nc.sync.dma_start(out=ws_t[:], in_=w_s[:])

            # ---------------- phase A: windowed scaled segment sums + proj
            for w in range(W_max):
                pd = pool.tile([128, T * 128], F32, tag="pd")
                nc.sync.dma_start(
                    out=pd[:].rearrange("p (t f) -> p t f", t=T),
                    in_=pay_d[w * T * 128 : (w + 1) * T * 128, :].rearrange(
                        "(t p) f -> p t f", p=128
                    ),
                )
                ps = pool.tile([128, T * 128], F32, tag="ps")
                nc.sync.dma_start(
                    out=ps[:].rearrange("p (t f) -> p t f", t=T),
                    in_=pay_s[w * T * 128 : (w + 1) * T * 128, :].rearrange(
                        "(t p) f -> p t f", p=128
                    ),
                )
                psw = pp.tile([128, 128], F32, tag="psw")
                for di, (pay_t, lo_t, wg_t) in enumerate(
                    ((pd, lo_d_t, wd_t), (ps, lo_s_t, ws_t))
                ):
                    for t in range(T):
                        col = w * T + t
                        oh = pool.tile([128, 128], F32, tag="oh")
                        nc.vector.tensor_scalar(
                            out=oh[:],
                            in0=iota_t[:],
                            scalar1=lo_t[:, col : col + 1],
                            scalar2=wg_t[:, col : col + 1],
                            op0=mybir.AluOpType.is_equal,
                            op1=mybir.AluOpType.mult,
                        )
                        nc.tensor.matmul(
                            out=psw[:],
                            lhsT=pay_t[:, t * 128 : (t + 1) * 128],
                            rhs=oh[:],
                            start=(di == 0 and t == 0),
                            stop=(di == 1 and t == T - 1),
                        )
                nfT = pool.tile([128, 128], F32, tag="nfT")
                nc.vector.tensor_copy(out=nfT[:], in_=psw[:])
                for tag, w1_t, b_t, loc in (
                    ("ph", wh1_t, bh_t, nph_loc),
                    ("pt", wt1_t, bt_t, npt_loc),
                ):
                    pj = pp.tile([128, 128], F32, tag=tag)
                    nc.tensor.matmul(
                        out=pj[:], lhsT=nfT[:], rhs=w1_t[:], start=True, stop=False
                    )
                    nc.tensor.matmul(
                        out=pj[:],
                        lhsT=b1_t[:1, w * 128 : (w + 1) * 128],
                        rhs=b_t[:1, :],
                        start=False,
                        stop=True,
                    )
                    st = pool.tile([128, 128], F32, tag=tag + "s")
                    nc.vector.tensor_copy(out=st[:], in_=pj[:])
                    nc.sync.dma_start(
                        out=loc[w * 128 : (w + 1) * 128, :], in_=st[:]
                    )

            # ---------------- phase B: all-gather projected node tables
            nc.gpsimd.collective_compute(
                "AllGather",
                mybir.AluOpType.bypass,
                replica_groups=rg,
                ins=[nph_loc[:]],
                outs=[nph_gl[:]],
            )
            nc.gpsimd.collective_compute(
                "AllGather",
                mybir.AluOpType.bypass,
                replica_groups=rg,
                ins=[npt_loc[:]],
                outs=[npt_gl[:]],
            )

            # ---------------- phase C: per-edge GEMM + node-row gather
            lo_view_h = nph_gl[: min(R, 32768), :]
            lo_view_t = npt_gl[: min(R, 32768), :]
            for e0, ch in chunks:
                nt = -(-ch // 128)
                ntc = nt * 128
                c0 = e0 // 16
                c1 = (e0 + ch) // 16
                hT = pool.tile([128, CH], F32, tag="hT")
                nc.sync.dma_start(out=hT[:, :ch], in_=efT[0:D, e0 : e0 + ch])
                tT = pool.tile([128, CH], F32, tag="tT")
                nc.sync.dma_start(out=tT[:, :ch], in_=efT[D : 2 * D, e0 : e0 + ch])

                passes = [("lo_h", lo_view_h, ixlo_h), ("lo_t", lo_view_t, ixlo_t)]
                if two_pass:
                    passes += [
                        ("hi_h", nph_gl[hibase : hibase + 32768, :], ixhi_h),
                        ("hi_t", npt_gl[hibase : hibase + 32768, :], ixhi_t),
                    ]
                gdst = {}
                for key, view, ix_dram in passes:
                    ix_t = pool.tile([128, CH // 16], I16, tag="ix" + key)
                    nc.sync.dma_start(
                        out=ix_t[:, : ch // 16], in_=ix_dram[:, c0:c1]
                    )
                    g = pool.tile([128, (CH // 128) * 128], F32, tag="g" + key)
                    nc.gpsimd.dma_gather(
                        out_ap=g[:, :ntc].rearrange("p (n d) -> p n d", d=D),
                        in_ap=view,
                        idxs_ap=ix_t[:, : ch // 16],
                        num_idxs=ch,
                        num_idxs_reg=ch,
                        elem_size=D,
                        elem_step=D,
                        single_packet=False,
                    )
                    gdst[key] = g

                ost = pool.tile([128, (CH // 128) * 2 * D], F32, tag="ost")
                for t in range(nt):
                    m = min(128, ch - t * 128)
                    for si, (eT, w2_t, lo_key, hi_key) in enumerate(
                        (
                            (hT, wh2_t, "lo_h", "hi_h"),
                            (tT, wt2_t, "lo_t", "hi_t"),
                        )
                    ):
                        pj = pp.tile([128, 128], F32, tag="pc")
                        nc.tensor.matmul(
                            out=pj[:m, :],
                            lhsT=eT[:, t * 128 : t * 128 + m],
                            rhs=w2_t[:],
                            start=True,
                            stop=True,
                        )
                        o_sl = ost[:m, t * 2 * D + si * D : t * 2 * D + (si + 1) * D]
                        nc.vector.tensor_tensor(
                            out=o_sl,
                            in0=pj[:m, :],
                            in1=gdst[lo_key][:m, t * D : (t + 1) * D],
                            op=mybir.AluOpType.add,
                        )
                        if two_pass:
                            nc.vector.tensor_tensor(
                                out=o_sl,
                                in0=o_sl,
                                in1=gdst[hi_key][:m, t * D : (t + 1) * D],
                                op=mybir.AluOpType.add,
                            )
                if ch % 128 == 0:
                    nc.sync.dma_start(
                        out=out[e0 : e0 + ch, :].rearrange(
                            "(t p) f -> p t f", p=128
                        ),
                        in_=ost[:, : nt * 2 * D].rearrange(
                            "p (t f) -> p t f", t=nt
                        ),
                    )
                else:
                    for t in range(nt):
                        m = min(128, ch - t * 128)
                        nc.sync.dma_start(
                            out=out[e0 + t * 128 : e0 + t * 128 + m, :],
                            in_=ost[:m, t * 2 * D : (t + 1) * 2 * D],
                        )

    nc.compile()
    return nc


# ------------------------------------------------------------------ driver

def _install_ntff_hook():
    """The agent image's antenv lacks axon_hooks; synthesize it so
    run_bass_kernel_spmd(trace=True) can capture NTFF profiles."""
    import types

    try:
        import antenv.axon_hooks  # noqa: F401

        return True
    except ImportError:
        pass
    try:
        import antenv
        from trn_agent_boot.trn_boot import _ntff_profile_via_ctypes

        hook = _ntff_profile_via_ctypes("/opt/axon/libaxon_pjrt.so")
        mod = types.ModuleType("antenv.axon_hooks")
        _state = {"hook": hook}
        mod.set_axon_ntff_profile_hook = lambda h: _state.update(hook=h)
        mod.get_axon_ntff_profile_hook = lambda: _state["hook"]
        sys.modules["antenv.axon_hooks"] = mod
        antenv.axon_hooks = mod
        return hook is not None
    except Exception:
        return False


_CACHE = {}


def _get_program(meta):
    key = (meta["E_c"], meta["W_max"], meta["R"], meta["two_pass"])
    if key not in _CACHE:
        _CACHE[key] = _build(meta)
    return _CACHE[key]


def kernel(
    efeat,
    src,
    dst,
    num_nodes,
    W_head,
    b_head,
    W_tail,
    b_tail,
    _trace=False,
):
    efeat = np.asarray(efeat, dtype=np.float32)
    src = np.asarray(src)
    dst = np.asarray(dst)
    N = int(num_nodes)
    in_maps, meta = _prep(
        efeat,
        src,
        dst,
        N,
        np.asarray(W_head, dtype=np.float32),
        np.asarray(b_head, dtype=np.float32),
        np.asarray(W_tail, dtype=np.float32),
        np.asarray(b_tail, dtype=np.float32),
    )
    nc = _get_program(meta)
    if _trace:
        _install_ntff_hook()
    res = run_bass_kernel_spmd(
        nc, in_maps, core_ids=list(range(N_CORES)), trace=_trace
    )
    out = np.concatenate([res.results[c]["out"] for c in range(N_CORES)], axis=0)
    if _trace:
        return out, res
    return out


# revision 8
# speedup vs baseline: 1.1283x; 1.1283x over previous
"""Bass/Trainium2 kernel for nn_DirectedLayer (GNN message passing).

Computes, for a directed graph with E edges and N nodes:
    head, tail = split(efeat, 2, axis=-1)
    mean_in  = segment_mean(head, dst, N)
    mean_out = segment_mean(tail, src, N)
    nfeat = 0.5 * (mean_in + mean_out)
    out = concat([concat([nfeat[src], head]) @ W_head + b_head,
                  concat([nfeat[dst], tail]) @ W_tail + b_tail], axis=-1)

Distribution over 8 NeuronCores:
  Phase A: nodes sharded by contiguous windows (<=128 nodes each); each core
           computes the scaled segment sums for its windows via one-hot
           matmuls over host-sorted edge payloads, then projects them through
           the first half of each weight matrix (bias folded in).
  Phase B: AllGather of the two projected node tables.
  Phase C: edges sharded contiguously; each core streams its transposed edge
           features, multiplies by the second half of the weights, gathers the
           projected node rows with 2-pass int16 dma_gather, adds, and stores.
"""

import sys

for p in ("/opt/trn_rl_repo/concourse", "/opt/trn_rl_repo"):
    if p not in sys.path:
        sys.path.insert(0, p)

import numpy as np

import concourse.bass as bass
import concourse.bacc as bacc
import concourse.mybir as mybir
import concourse.tile as tile
from concourse.bass_utils import run_bass_kernel_spmd

N_CORES = 8
D = 128          # feature dim per half
T = 9            # edge tiles (of 128) per window, per direction
CH = 2048        # phase-C chunk size in edges
F32 = mybir.dt.float32
I16 = mybir.dt.int16


# ---------------------------------------------------------------- host prep

def _window_partition(cin, cout, N):
    """Greedy split of nodes 0..N-1 into consecutive windows with
    <=128 nodes (127 for window 0; slot 0 is a reserved zero row) and
    <=T*128 incident edges per direction."""
    cap_e = T * 128
    w_id = np.empty(N, dtype=np.int64)
    slot = np.empty(N, dtype=np.int64)
    w = 0
    nn = 0
    de = 0
    se = 0
    first_slot = 1  # window 0 reserves slot 0
    for n in range(N):
        cap_n = 128 - (1 if w == 0 else 0)
        if nn > 0 and (
            nn >= cap_n or de + cin[n] > cap_e or se + cout[n] > cap_e
        ):
            w += 1
            nn = 0
            de = 0
            se = 0
            first_slot = 0
        w_id[n] = w
        slot[n] = nn + (first_slot if w == 0 else 0)
        nn += 1
        de += cin[n]
        se += cout[n]
    return w_id, slot, w + 1


def _dir_payload(efeat_half, nidx, wgt_node, w_id, slot, core, W_max, order):
    """Payload/one-hot metadata for one direction on one core.

    nidx: per-edge node index (dst or src); order: edges sorted by w_id[nidx].
    Returns (pay [W_max*T*128, D], loff [128, W_max*T], wcol [128, W_max*T]).
    """
    w_of_e = w_id[nidx[order]]
    lo_w = core * W_max
    hi_w = (core + 1) * W_max
    m = (w_of_e >= lo_w) & (w_of_e < hi_w)
    ed = order[m]
    we = w_of_e[m] - lo_w
    # rank of each edge within its window (edges are grouped by window)
    starts = np.searchsorted(we, np.arange(W_max))
    rank = np.arange(len(ed)) - starts[we]
    pos = we * (T * 128) + rank
    assert rank.max(initial=0) < T * 128

    rows = W_max * T * 128
    pay = np.zeros((rows, D), dtype=np.float32)
    loff = np.zeros(rows, dtype=np.float32)
    wcol = np.zeros(rows, dtype=np.float32)
    pay[pos] = efeat_half[ed]
    loff[pos] = slot[nidx[ed]]
    wcol[pos] = wgt_node[nidx[ed]]
    return (
        pay,
        np.ascontiguousarray(loff.reshape(-1, 128).T),
        np.ascontiguousarray(wcol.reshape(-1, 128).T),
    )


def _wrap_idx(gidx, chunks):
    """Pack per-chunk wrapped int16 index table [128, ceil(E_c/16)]."""
    cols = sum(ch for _, ch in chunks) // 16
    out = np.empty((16, cols), dtype=np.int16)
    c0 = 0
    for e0, ch in chunks:
        out[:, c0 : c0 + ch // 16] = (
            gidx[e0 : e0 + ch].reshape(ch // 16, 16).T
        )
        c0 += ch // 16
    return np.ascontiguousarray(np.tile(out, (8, 1)))


def _chunks(E_c):
    out = []
    e0 = 0
    while e0 < E_c:
        ch = min(CH, E_c - e0)
        if ch % 16:
            # keep every chunk a multiple of 16 except never needed for our E
            raise ValueError("edge shard must be a multiple of 16")
        out.append((e0, ch))
        e0 += ch
    return out


def _prep(efeat, src, dst, N, W_head, b_head, W_tail, b_tail):
    E = src.shape[0]
    assert E % N_CORES == 0
    E_c = E // N_CORES
    src = src.astype(np.int64)
    dst = dst.astype(np.int64)

    cin = np.bincount(dst, minlength=N)
    cout = np.bincount(src, minlength=N)
    w_in = (0.5 / np.maximum(cin, 1)).astype(np.float32)
    w_out = (0.5 / np.maximum(cout, 1)).astype(np.float32)

    w_id, slot, W_total = _window_partition(cin, cout, N)
    W_max = -(-W_total // N_CORES)
    # row R-1 must stay a zero row (junk target of the hi gather pass)
    if W_total == N_CORES * W_max:
        last = np.where(w_id == W_total - 1)[0]
        if slot[last].max() >= 127:
            W_max += 1
    R = N_CORES * W_max * 128
    assert R <= 65536, f"node table too large for 2-pass int16 gather: {R}"
    two_pass = R > 32768
    hibase = R - 32768

    row = w_id * 128 + slot  # global row of each node in the gathered tables

    order_d = np.argsort(w_id[dst], kind="stable")
    order_s = np.argsort(w_id[src], kind="stable")

    gh = row[src]  # phase-C gather rows for the head output (nfeat[src])
    gt = row[dst]
    chunks = _chunks(E_c)

    iota = np.tile(np.arange(128, dtype=np.float32), (128, 1))

    in_maps = []
    for c in range(N_CORES):
        pay_d, loff_d, w_d = _dir_payload(
            efeat[:, :D], dst, w_in, w_id, slot, c, W_max, order_d
        )
        pay_s, loff_s, w_s = _dir_payload(
            efeat[:, D:], src, w_out, w_id, slot, c, W_max, order_s
        )
        bias1 = np.zeros((1, W_max * 128), dtype=np.float32)
        mine = (w_id >= c * W_max) & (w_id < (c + 1) * W_max)
        bias1[0, (w_id[mine] - c * W_max) * 128 + slot[mine]] = 1.0

        sl = slice(c * E_c, (c + 1) * E_c)
        ghc = gh[sl]
        gtc = gt[sl]
        if two_pass:
            glo_h = np.where(ghc < 32768, ghc, 0).astype(np.int16)
            ghi_h = np.where(ghc >= 32768, ghc - hibase, 32767).astype(np.int16)
            glo_t = np.where(gtc < 32768, gtc, 0).astype(np.int16)
            ghi_t = np.where(gtc >= 32768, gtc - hibase, 32767).astype(np.int16)
        else:
            glo_h = ghc.astype(np.int16)
            ghi_h = glo_t = ghi_t = None
            glo_t = gtc.astype(np.int16)

        m = {
            "pay_d": pay_d,
            "pay_s": pay_s,
            "loff_d": loff_d,
            "loff_s": loff_s,
            "w_d": w_d,
            "w_s": w_s,
            "bias1": bias1,
            "iota": iota,
            "Wh1": np.ascontiguousarray(W_head[:D]),
            "Wh2": np.ascontiguousarray(W_head[D:]),
            "Wt1": np.ascontiguousarray(W_tail[:D]),
            "Wt2": np.ascontiguousarray(W_tail[D:]),
            "bh": b_head.reshape(1, D).astype(np.float32),
            "bt": b_tail.reshape(1, D).astype(np.float32),
            "efT": np.ascontiguousarray(efeat[sl].T),
            "ixlo_h": _wrap_idx(glo_h, chunks),
            "ixlo_t": _wrap_idx(glo_t, chunks),
        }
        if two_pass:
            m["ixhi_h"] = _wrap_idx(ghi_h, chunks)
            m["ixhi_t"] = _wrap_idx(ghi_t, chunks)
        in_maps.append(m)

    meta = {
        "E_c": E_c,
        "W_max": W_max,
        "R": R,
        "two_pass": two_pass,
        "hibase": hibase,
        "chunks": chunks,
        "idx_cols": sum(ch for _, ch in chunks) // 16,
    }
    return in_maps, meta


# ------------------------------------------------------------- device build

def _build(meta):
    E_c = meta["E_c"]
    W_max = meta["W_max"]
    R = meta["R"]
    two_pass = meta["two_pass"]
    hibase = meta["hibase"]
    chunks = meta["chunks"]
    icols = meta["idx_cols"]
    WT = W_max * T

    nc = bacc.Bacc(None, num_devices=N_CORES, debug=False, num_swdge_queues=4)

    pay_d = nc.dram_tensor("pay_d", [WT * 128, D], F32, kind="ExternalInput")
    pay_s = nc.dram_tensor("pay_s", [WT * 128, D], F32, kind="ExternalInput")
    loff_d = nc.dram_tensor("loff_d", [128, WT], F32, kind="ExternalInput")
    loff_s = nc.dram_tensor("loff_s", [128, WT], F32, kind="ExternalInput")
    w_d = nc.dram_tensor("w_d", [128, WT], F32, kind="ExternalInput")
    w_s = nc.dram_tensor("w_s", [128, WT], F32, kind="ExternalInput")
    bias1 = nc.dram_tensor("bias1", [1, W_max * 128], F32, kind="ExternalInput")
    iota_in = nc.dram_tensor("iota", [128, 128], F32, kind="ExternalInput")
    Wh1 = nc.dram_tensor("Wh1", [D, D], F32, kind="ExternalInput")
    Wh2 = nc.dram_tensor("Wh2", [D, D], F32, kind="ExternalInput")
    Wt1 = nc.dram_tensor("Wt1", [D, D], F32, kind="ExternalInput")
    Wt2 = nc.dram_tensor("Wt2", [D, D], F32, kind="ExternalInput")
    bh = nc.dram_tensor("bh", [1, D], F32, kind="ExternalInput")
    bt = nc.dram_tensor("bt", [1, D], F32, kind="ExternalInput")
    efT = nc.dram_tensor("efT", [2 * D, E_c], F32, kind="ExternalInput")
    ixlo_h = nc.dram_tensor("ixlo_h", [128, icols], I16, kind="ExternalInput")
    ixlo_t = nc.dram_tensor("ixlo_t", [128, icols], I16, kind="ExternalInput")
    if two_pass:
        ixhi_h = nc.dram_tensor("ixhi_h", [128, icols], I16, kind="ExternalInput")
        ixhi_t = nc.dram_tensor("ixhi_t", [128, icols], I16, kind="ExternalInput")
    out = nc.dram_tensor("out", [E_c, 2 * D], F32, kind="ExternalOutput")

    nph_loc = nc.dram_tensor("nph_loc", [W_max * 128, D], F32)
    npt_loc = nc.dram_tensor("npt_loc", [W_max * 128, D], F32)
    nph_gl = nc.dram_tensor("nph_gl", [R, D], F32, addr_space="Shared")
    npt_gl = nc.dram_tensor("npt_gl", [R, D], F32, addr_space="Shared")

    rg = [list(range(N_CORES))]

    with tile.TileContext(nc) as tc:
        with (
            tc.tile_pool(name="const", bufs=1) as cpool,
            tc.tile_pool(name="sbuf", bufs=2) as pool,
            tc.tile_pool(name="psum", bufs=2, space="PSUM") as pp,
        ):
            iota_t = cpool.tile([128, 128], F32, tag="iota")
            nc.sync.dma_start(out=iota_t[:], in_=iota_in[:])
            wh1_t = cpool.tile([D, D], F32, tag="wh1")
            nc.sync.dma_start(out=wh1_t[:], in_=Wh1[:])
            wh2_t = cpool.tile([D, D], F32, tag="wh2")
            nc.sync.dma_start(out=wh2_t[:], in_=Wh2[:])
            wt1_t = cpool.tile([D, D], F32, tag="wt1")
            nc.sync.dma_start(out=wt1_t[:], in_=Wt1[:])
            wt2_t = cpool.tile([D, D], F32, tag="wt2")
            nc.sync.dma_start(out=wt2_t[:], in_=Wt2[:])
            bh_t = cpool.tile([1, D], F32, tag="bh")
            nc.sync.dma_start(out=bh_t[:], in_=bh[:])
            bt_t = cpool.tile([1, D], F32, tag="bt")
            nc.sync.dma_start(out=bt_t[:], in_=bt[:])
            b1_t = cpool.tile([1, W_max * 128], F32, tag="b1")
            nc.sync.dma_start(out=b1_t[:], in_=bias1[:])
            lo_d_t = cpool.tile([128, WT], F32, tag="lod")
            nc.sync.dma_start(out=lo_d_t[:], in_=loff_d[:])
            lo_s_t = cpool.tile([128, WT], F32, tag="los")
            nc.sync.dma_start(out=lo_s_t[:], in_=loff_s[:])
            wd_t = cpool.tile([128, WT], F32, tag="wd")
            nc.sync.dma_start(out=wd_t[:], in_=w_d[:])
            ws_t = cpool.tile([128, WT], F32, tag="ws")
            nc.sync.dma_start(out=ws_t[:], in_=w_s[:])

            # ---------------- phase A: windowed scaled segment sums + proj
            for w in range(W_max):
                pd = pool.tile([128, T * 128], F32, tag="pd")
                nc.sync.dma_start(
                    out=pd[:].rearrange("p (t f) -> p t f", t=T),
                    in_=pay_d[w * T * 128 : (w + 1) * T * 128, :].rearrange(
                        "(t p) f -> p t f", p=128
                    ),
                )
                ps = pool.tile([128, T * 128], F32, tag="ps")
                nc.sync.dma_start(
                    out=ps[:].rearrange("p (t f) -> p t f", t=T),
                    in_=pay_s[w * T * 128 : (w + 1) * T * 128, :].rearrange(
                        "(t p) f -> p t f", p=128
                    ),
                )
                psw = pp.tile([128, 128], F32, tag="psw")
                for di, (pay_t, lo_t, wg_t) in enumerate(
                    ((pd, lo_d_t, wd_t), (ps, lo_s_t, ws_t))
                ):
                    for t in range(T):
                        col = w * T + t
                        oh = pool.tile([128, 128], F32, tag="oh")
                        nc.any.tensor_scalar(
                            out=oh[:],
                            in0=iota_t[:],
                            scalar1=lo_t[:, col : col + 1],
                            scalar2=wg_t[:, col : col + 1],
                            op0=mybir.AluOpType.is_equal,
                            op1=mybir.AluOpType.mult,
                        )
                        nc.tensor.matmul(
                            out=psw[:],
                            lhsT=pay_t[:, t * 128 : (t + 1) * 128],
                            rhs=oh[:],
                            start=(di == 0 and t == 0),
                            stop=(di == 1 and t == T - 1),
                        )
                nfT = pool.tile([128, 128], F32, tag="nfT")
                nc.any.tensor_copy(out=nfT[:], in_=psw[:])
                for tag, w1_t, b_t, loc in (
                    ("ph", wh1_t, bh_t, nph_loc),
                    ("pt", wt1_t, bt_t, npt_loc),
                ):
                    pj = pp.tile([128, 128], F32, tag=tag)
                    nc.tensor.matmul(
                        out=pj[:], lhsT=nfT[:], rhs=w1_t[:], start=True, stop=False
                    )
                    nc.tensor.matmul(
                        out=pj[:],
                        lhsT=b1_t[:1, w * 128 : (w + 1) * 128],
                        rhs=b_t[:1, :],
                        start=False,
                        stop=True,
                    )
                    st = pool.tile([128, 128], F32, tag=tag + "s")
                    nc.any.tensor_copy(out=st[:], in_=pj[:])
                    nc.sync.dma_start(
                        out=loc[w * 128 : (w + 1) * 128, :], in_=st[:]
                    )

            # ---------------- phase B: all-gather projected node tables
            nc.gpsimd.collective_compute(
                "AllGather",
                mybir.AluOpType.bypass,
                replica_groups=rg,
                ins=[nph_loc[:]],
                outs=[nph_gl[:]],
            )
            nc.gpsimd.collective_compute(
                "AllGather",
                mybir.AluOpType.bypass,
                replica_groups=rg,
                ins=[npt_loc[:]],
                outs=[npt_gl[:]],
            )

            # ---------------- phase C: per-edge GEMM + node-row gather
            lo_view_h = nph_gl[: min(R, 32768), :]
            lo_view_t = npt_gl[: min(R, 32768), :]
            for e0, ch in chunks:
                nt = -(-ch // 128)
                ntc = nt * 128
                c0 = e0 // 16
                c1 = (e0 + ch) // 16
                hT = pool.tile([128, CH], F32, tag="hT")
                nc.sync.dma_start(out=hT[:, :ch], in_=efT[0:D, e0 : e0 + ch])
                tT = pool.tile([128, CH], F32, tag="tT")
                nc.sync.dma_start(out=tT[:, :ch], in_=efT[D : 2 * D, e0 : e0 + ch])

                passes = [
                    ("lo_h", lo_view_h, ixlo_h, 0),
                    ("lo_t", lo_view_t, ixlo_t, 1),
                ]
                if two_pass:
                    passes += [
                        ("hi_h", nph_gl[hibase : hibase + 32768, :], ixhi_h, 2),
                        ("hi_t", npt_gl[hibase : hibase + 32768, :], ixhi_t, 3),
                    ]
                gdst = {}
                for key, view, ix_dram, qn in passes:
                    ix_t = pool.tile([128, CH // 16], I16, tag="ix" + key)
                    nc.sync.dma_start(
                        out=ix_t[:, : ch // 16], in_=ix_dram[:, c0:c1]
                    )
                    g = pool.tile([128, (CH // 128) * 128], F32, tag="g" + key)
                    nc.gpsimd.dma_gather(
                        out_ap=g[:, :ntc].rearrange("p (n d) -> p n d", d=D),
                        in_ap=view,
                        idxs_ap=ix_t[:, : ch // 16],
                        num_idxs=ch,
                        num_idxs_reg=ch,
                        elem_size=D,
                        elem_step=D,
                        single_packet=False,
                        queue_num=qn,
                    )
                    gdst[key] = g

                ost = pool.tile([128, (CH // 128) * 2 * D], F32, tag="ost")
                for t in range(nt):
                    m = min(128, ch - t * 128)
                    for si, (eT, w2_t, lo_key, hi_key) in enumerate(
                        (
                            (hT, wh2_t, "lo_h", "hi_h"),
                            (tT, wt2_t, "lo_t", "hi_t"),
                        )
                    ):
                        pj = pp.tile([128, 128], F32, tag="pc")
                        nc.tensor.matmul(
                            out=pj[:m, :],
                            lhsT=eT[:, t * 128 : t * 128 + m],
                            rhs=w2_t[:],
                            start=True,
                            stop=True,
                        )
                        o_sl = ost[:m, t * 2 * D + si * D : t * 2 * D + (si + 1) * D]
                        nc.any.tensor_tensor(
                            out=o_sl,
                            in0=pj[:m, :],
                            in1=gdst[lo_key][:m, t * D : (t + 1) * D],
                            op=mybir.AluOpType.add,
                        )
                        if two_pass:
                            nc.any.tensor_tensor(
                                out=o_sl,
                                in0=o_sl,
                                in1=gdst[hi_key][:m, t * D : (t + 1) * D],
                                op=mybir.AluOpType.add,
                            )
                if ch % 128 == 0:
                    nc.sync.dma_start(
                        out=out[e0 : e0 + ch, :].rearrange(
                            "(t p) f -> p t f", p=128
                        ),
                        in_=ost[:, : nt * 2 * D].rearrange(
                            "p (t f) -> p t f", t=nt
                        ),
                    )
                else:
                    for t in range(nt):
                        m = min(128, ch - t * 128)
                        nc.sync.dma_start(
                            out=out[e0 + t * 128 : e0 + t * 128 + m, :],
                            in_=ost[:m, t * 2 * D : (t + 1) * 2 * D],
                        )

    nc.compile()
    return nc


# ------------------------------------------------------------------ driver

def _install_ntff_hook():
    """The agent image's antenv lacks axon_hooks; synthesize it so
    run_bass_kernel_spmd(trace=True) can capture NTFF profiles."""
    import types

    try:
        import antenv.axon_hooks  # noqa: F401

        return True
    except ImportError:
        pass
    try:
        import antenv
        from trn_agent_boot.trn_boot import _ntff_profile_via_ctypes

        hook = _ntff_profile_via_ctypes("/opt/axon/libaxon_pjrt.so")
        mod = types.ModuleType("antenv.axon_hooks")
        _state = {"hook": hook}
        mod.set_axon_ntff_profile_hook = lambda h: _state.update(hook=h)
        mod.get_axon_ntff_profile_hook = lambda: _state["hook"]
        sys.modules["antenv.axon_hooks"] = mod
        antenv.axon_hooks = mod
        return hook is not None
    except Exception:
        return False


_CACHE = {}


def _get_program(meta):
    key = (meta["E_c"], meta["W_max"], meta["R"], meta["two_pass"])
    if key not in _CACHE:
        _CACHE[key] = _build(meta)
    return _CACHE[key]


def kernel(
    efeat,
    src,
    dst,
    num_nodes,
    W_head,
    b_head,
    W_tail,
    b_tail,
    _trace=False,
):
    efeat = np.asarray(efeat, dtype=np.float32)
    src = np.asarray(src)
    dst = np.asarray(dst)
    N = int(num_nodes)
    in_maps, meta = _prep(
        efeat,
        src,
        dst,
        N,
        np.asarray(W_head, dtype=np.float32),
        np.asarray(b_head, dtype=np.float32),
        np.asarray(W_tail, dtype=np.float32),
        np.asarray(b_tail, dtype=np.float32),
    )
    nc = _get_program(meta)
    if _trace:
        _install_ntff_hook()
    res = run_bass_kernel_spmd(
        nc, in_maps, core_ids=list(range(N_CORES)), trace=_trace
    )
    out = np.concatenate([res.results[c]["out"] for c in range(N_CORES)], axis=0)
    if _trace:
        return out, res
    return out


# revision 11
# speedup vs baseline: 1.2151x; 1.0769x over previous
"""Bass/Trainium2 kernel for nn_DirectedLayer (GNN message passing).

Computes, for a directed graph with E edges and N nodes:
    head, tail = split(efeat, 2, axis=-1)
    mean_in  = segment_mean(head, dst, N)
    mean_out = segment_mean(tail, src, N)
    nfeat = 0.5 * (mean_in + mean_out)
    out = concat([concat([nfeat[src], head]) @ W_head + b_head,
                  concat([nfeat[dst], tail]) @ W_tail + b_tail], axis=-1)

Distribution over 8 NeuronCores:
  Phase A: nodes sharded by contiguous windows (<=128 nodes each); each core
           computes the scaled segment sums for its windows via one-hot
           matmuls over host-sorted edge payloads, then projects them through
           the first half of each weight matrix (bias folded in).
  Phase B: AllGather of the two projected node tables.
  Phase C: edges sharded contiguously; each core streams its transposed edge
           features, multiplies by the second half of the weights, gathers the
           projected node rows with 2-pass int16 dma_gather, adds, and stores.
"""

import sys

for p in ("/opt/trn_rl_repo/concourse", "/opt/trn_rl_repo"):
    if p not in sys.path:
        sys.path.insert(0, p)

import numpy as np

import concourse.bass as bass
import concourse.bacc as bacc
import concourse.mybir as mybir
import concourse.tile as tile
from concourse.bass_utils import run_bass_kernel_spmd

N_CORES = 8
D = 128          # feature dim per half
T = 9            # edge tiles (of 128) per window, per direction
CH = 2048        # phase-C chunk size in edges
PREC = "f32r"    # "fp32" | "f32r" | "bf16"
F32 = mybir.dt.float32
I16 = mybir.dt.int16


# ---------------------------------------------------------------- host prep

def _window_partition(cin, cout, N):
    """Greedy split of nodes 0..N-1 into consecutive windows with
    <=128 nodes (127 for window 0; slot 0 is a reserved zero row) and
    <=T*128 incident edges per direction."""
    cap_e = T * 128
    w_id = np.empty(N, dtype=np.int64)
    slot = np.empty(N, dtype=np.int64)
    w = 0
    nn = 0
    de = 0
    se = 0
    first_slot = 1  # window 0 reserves slot 0
    for n in range(N):
        cap_n = 128 - (1 if w == 0 else 0)
        if nn > 0 and (
            nn >= cap_n or de + cin[n] > cap_e or se + cout[n] > cap_e
        ):
            w += 1
            nn = 0
            de = 0
            se = 0
            first_slot = 0
        w_id[n] = w
        slot[n] = nn + (first_slot if w == 0 else 0)
        nn += 1
        de += cin[n]
        se += cout[n]
    return w_id, slot, w + 1


def _dir_payload(efeat_half, nidx, wgt_node, w_id, slot, core, W_max, order):
    """Payload/one-hot metadata for one direction on one core.

    nidx: per-edge node index (dst or src); order: edges sorted by w_id[nidx].
    Returns (pay [W_max*T*128, D], loff [128, W_max*T], wcol [128, W_max*T]).
    """
    w_of_e = w_id[nidx[order]]
    lo_w = core * W_max
    hi_w = (core + 1) * W_max
    m = (w_of_e >= lo_w) & (w_of_e < hi_w)
    ed = order[m]
    we = w_of_e[m] - lo_w
    # rank of each edge within its window (edges are grouped by window)
    starts = np.searchsorted(we, np.arange(W_max))
    rank = np.arange(len(ed)) - starts[we]
    pos = we * (T * 128) + rank
    assert rank.max(initial=0) < T * 128

    rows = W_max * T * 128
    pay = np.zeros((rows, D), dtype=np.float32)
    loff = np.zeros(rows, dtype=np.float32)
    wcol = np.zeros(rows, dtype=np.float32)
    pay[pos] = efeat_half[ed]
    loff[pos] = slot[nidx[ed]]
    wcol[pos] = wgt_node[nidx[ed]]
    pay_w = np.ascontiguousarray(
        pay.reshape(-1, 128, D).transpose(1, 0, 2).reshape(128, -1)
    )
    return (
        pay_w,
        np.ascontiguousarray(loff.reshape(-1, 128).T),
        np.ascontiguousarray(wcol.reshape(-1, 128).T),
    )


def _wrap_idx(gidx, chunks):
    """Pack per-chunk wrapped int16 index table [128, ceil(E_c/16)]."""
    cols = sum(ch for _, ch in chunks) // 16
    out = np.empty((16, cols), dtype=np.int16)
    c0 = 0
    for e0, ch in chunks:
        out[:, c0 : c0 + ch // 16] = (
            gidx[e0 : e0 + ch].reshape(ch // 16, 16).T
        )
        c0 += ch // 16
    return np.ascontiguousarray(np.tile(out, (8, 1)))


def _chunks(E_c):
    out = []
    e0 = 0
    while e0 < E_c:
        ch = min(CH, E_c - e0)
        if ch % 16:
            # keep every chunk a multiple of 16 except never needed for our E
            raise ValueError("edge shard must be a multiple of 16")
        out.append((e0, ch))
        e0 += ch
    return out


def _prep(efeat, src, dst, N, W_head, b_head, W_tail, b_tail):
    E = src.shape[0]
    assert E % N_CORES == 0
    E_c = E // N_CORES
    src = src.astype(np.int64)
    dst = dst.astype(np.int64)

    cin = np.bincount(dst, minlength=N)
    cout = np.bincount(src, minlength=N)
    w_in = (0.5 / np.maximum(cin, 1)).astype(np.float32)
    w_out = (0.5 / np.maximum(cout, 1)).astype(np.float32)

    w_id, slot, W_total = _window_partition(cin, cout, N)
    W_max = -(-W_total // N_CORES)
    # row R-1 must stay a zero row (junk target of the hi gather pass)
    if W_total == N_CORES * W_max:
        last = np.where(w_id == W_total - 1)[0]
        if slot[last].max() >= 127:
            W_max += 1
    R = N_CORES * W_max * 128
    assert R <= 65536, f"node table too large for 2-pass int16 gather: {R}"
    two_pass = R > 32768
    hibase = R - 32768

    row = w_id * 128 + slot  # global row of each node in the gathered tables

    order_d = np.argsort(w_id[dst], kind="stable")
    order_s = np.argsort(w_id[src], kind="stable")

    gh = row[src]  # phase-C gather rows for the head output (nfeat[src])
    gt = row[dst]
    chunks = _chunks(E_c)

    iota = np.tile(np.arange(128, dtype=np.float32), (128, 1))

    in_maps = []
    for c in range(N_CORES):
        pay_d, loff_d, w_d = _dir_payload(
            efeat[:, :D], dst, w_in, w_id, slot, c, W_max, order_d
        )
        pay_s, loff_s, w_s = _dir_payload(
            efeat[:, D:], src, w_out, w_id, slot, c, W_max, order_s
        )
        bias1 = np.zeros((1, W_max * 128), dtype=np.float32)
        mine = (w_id >= c * W_max) & (w_id < (c + 1) * W_max)
        bias1[0, (w_id[mine] - c * W_max) * 128 + slot[mine]] = 1.0

        sl = slice(c * E_c, (c + 1) * E_c)
        ghc = gh[sl]
        gtc = gt[sl]
        if two_pass:
            glo_h = np.where(ghc < 32768, ghc, 0).astype(np.int16)
            ghi_h = np.where(ghc >= 32768, ghc - hibase, 32767).astype(np.int16)
            glo_t = np.where(gtc < 32768, gtc, 0).astype(np.int16)
            ghi_t = np.where(gtc >= 32768, gtc - hibase, 32767).astype(np.int16)
        else:
            glo_h = ghc.astype(np.int16)
            ghi_h = glo_t = ghi_t = None
            glo_t = gtc.astype(np.int16)

        m = {
            "pay_d": pay_d,
            "pay_s": pay_s,
            "loff_d": loff_d,
            "loff_s": loff_s,
            "w_d": w_d,
            "w_s": w_s,
            "bias1": bias1,
            "iota": iota,
            "Wh1": np.ascontiguousarray(W_head[:D]),
            "Wh2": np.ascontiguousarray(W_head[D:]),
            "Wt1": np.ascontiguousarray(W_tail[:D]),
            "Wt2": np.ascontiguousarray(W_tail[D:]),
            "bh": b_head.reshape(1, D).astype(np.float32),
            "bt": b_tail.reshape(1, D).astype(np.float32),
            "efT": np.ascontiguousarray(efeat[sl].T),
            "ixlo_h": _wrap_idx(glo_h, chunks),
            "ixlo_t": _wrap_idx(glo_t, chunks),
        }
        if two_pass:
            m["ixhi_h"] = _wrap_idx(ghi_h, chunks)
            m["ixhi_t"] = _wrap_idx(ghi_t, chunks)
        in_maps.append(m)

    meta = {
        "E_c": E_c,
        "W_max": W_max,
        "R": R,
        "two_pass": two_pass,
        "hibase": hibase,
        "chunks": chunks,
        "idx_cols": sum(ch for _, ch in chunks) // 16,
    }
    return in_maps, meta


# ------------------------------------------------------------- device build

def _build(meta):
    E_c = meta["E_c"]
    W_max = meta["W_max"]
    R = meta["R"]
    two_pass = meta["two_pass"]
    hibase = meta["hibase"]
    chunks = meta["chunks"]
    icols = meta["idx_cols"]
    WT = W_max * T
    CHB = CH // 128

    if PREC == "fp32":
        DT = F32
    elif PREC == "f32r":
        DT = mybir.dt.float32r
    else:
        DT = mybir.dt.bfloat16
    TDT = mybir.dt.bfloat16 if PREC == "bf16" else F32  # node-table dtype

    nc = bacc.Bacc(None, num_devices=N_CORES, debug=False, num_swdge_queues=4)

    pay_d = nc.dram_tensor("pay_d", [128, WT * D], DT, kind="ExternalInput")
    pay_s = nc.dram_tensor("pay_s", [128, WT * D], DT, kind="ExternalInput")
    loff_d = nc.dram_tensor("loff_d", [128, WT], F32, kind="ExternalInput")
    loff_s = nc.dram_tensor("loff_s", [128, WT], F32, kind="ExternalInput")
    w_d = nc.dram_tensor("w_d", [128, WT], F32, kind="ExternalInput")
    w_s = nc.dram_tensor("w_s", [128, WT], F32, kind="ExternalInput")
    bias1 = nc.dram_tensor("bias1", [1, W_max * 128], DT, kind="ExternalInput")
    iota_in = nc.dram_tensor("iota", [128, 128], DT, kind="ExternalInput")
    Wh1 = nc.dram_tensor("Wh1", [D, D], DT, kind="ExternalInput")
    Wh2 = nc.dram_tensor("Wh2", [D, D], DT, kind="ExternalInput")
    Wt1 = nc.dram_tensor("Wt1", [D, D], DT, kind="ExternalInput")
    Wt2 = nc.dram_tensor("Wt2", [D, D], DT, kind="ExternalInput")
    bh = nc.dram_tensor("bh", [1, D], DT, kind="ExternalInput")
    bt = nc.dram_tensor("bt", [1, D], DT, kind="ExternalInput")
    efT = nc.dram_tensor("efT", [2 * D, E_c], DT, kind="ExternalInput")
    ixlo_h = nc.dram_tensor("ixlo_h", [128, icols], I16, kind="ExternalInput")
    ixlo_t = nc.dram_tensor("ixlo_t", [128, icols], I16, kind="ExternalInput")
    if two_pass:
        ixhi_h = nc.dram_tensor("ixhi_h", [128, icols], I16, kind="ExternalInput")
        ixhi_t = nc.dram_tensor("ixhi_t", [128, icols], I16, kind="ExternalInput")
    outw = nc.dram_tensor(
        "outw", [len(chunks) * 128, CHB * 2 * D], F32, kind="ExternalOutput"
    )

    nph_loc = nc.dram_tensor("nph_loc", [W_max * 128, D], TDT)
    npt_loc = nc.dram_tensor("npt_loc", [W_max * 128, D], TDT)
    nph_gl = nc.dram_tensor("nph_gl", [R, D], TDT, addr_space="Shared")
    npt_gl = nc.dram_tensor("npt_gl", [R, D], TDT, addr_space="Shared")

    rg = [list(range(N_CORES))]

    with tile.TileContext(nc) as tc:
        with (
            tc.tile_pool(name="const", bufs=1) as cpool,
            tc.tile_pool(name="sbuf", bufs=2) as pool,
            tc.tile_pool(name="psum", bufs=2, space="PSUM") as pp,
        ):
            iota_t = cpool.tile([128, 128], DT, tag="iota")
            nc.sync.dma_start(out=iota_t[:], in_=iota_in[:])
            wh1_t = cpool.tile([D, D], DT, tag="wh1")
            nc.sync.dma_start(out=wh1_t[:], in_=Wh1[:])
            wh2_t = cpool.tile([D, D], DT, tag="wh2")
            nc.sync.dma_start(out=wh2_t[:], in_=Wh2[:])
            wt1_t = cpool.tile([D, D], DT, tag="wt1")
            nc.sync.dma_start(out=wt1_t[:], in_=Wt1[:])
            wt2_t = cpool.tile([D, D], DT, tag="wt2")
            nc.sync.dma_start(out=wt2_t[:], in_=Wt2[:])
            bh_t = cpool.tile([1, D], DT, tag="bh")
            nc.sync.dma_start(out=bh_t[:], in_=bh[:])
            bt_t = cpool.tile([1, D], DT, tag="bt")
            nc.sync.dma_start(out=bt_t[:], in_=bt[:])
            b1_t = cpool.tile([1, W_max * 128], DT, tag="b1")
            nc.sync.dma_start(out=b1_t[:], in_=bias1[:])
            lo_d_t = cpool.tile([128, WT], F32, tag="lod")
            nc.sync.dma_start(out=lo_d_t[:], in_=loff_d[:])
            lo_s_t = cpool.tile([128, WT], F32, tag="los")
            nc.sync.dma_start(out=lo_s_t[:], in_=loff_s[:])
            wd_t = cpool.tile([128, WT], F32, tag="wd")
            nc.sync.dma_start(out=wd_t[:], in_=w_d[:])
            ws_t = cpool.tile([128, WT], F32, tag="ws")
            nc.sync.dma_start(out=ws_t[:], in_=w_s[:])

            # ---------------- phase A: windowed scaled segment sums + proj
            for w in range(W_max):
                pd = pool.tile([128, T * D], DT, tag="pd")
                nc.sync.dma_start(
                    out=pd[:], in_=pay_d[:, w * T * D : (w + 1) * T * D]
                )
                ps = pool.tile([128, T * D], DT, tag="ps")
                nc.sync.dma_start(
                    out=ps[:], in_=pay_s[:, w * T * D : (w + 1) * T * D]
                )
                psw = pp.tile([128, 128], F32, tag="psw")
                for di, (pay_t, lo_t, wg_t) in enumerate(
                    ((pd, lo_d_t, wd_t), (ps, lo_s_t, ws_t))
                ):
                    for t in range(T):
                        col = w * T + t
                        oh = pool.tile([128, 128], DT, tag="oh")
                        nc.any.tensor_scalar(
                            out=oh[:],
                            in0=iota_t[:],
                            scalar1=lo_t[:, col : col + 1],
                            scalar2=wg_t[:, col : col + 1],
                            op0=mybir.AluOpType.is_equal,
                            op1=mybir.AluOpType.mult,
                        )
                        nc.tensor.matmul(
                            out=psw[:],
                            lhsT=pay_t[:, t * D : (t + 1) * D],
                            rhs=oh[:],
                            start=(di == 0 and t == 0),
                            stop=(di == 1 and t == T - 1),
                        )
                nfT = pool.tile([128, 128], DT, tag="nfT")
                nc.any.tensor_copy(out=nfT[:], in_=psw[:])
                for tag, w1_t, b_t, loc in (
                    ("ph", wh1_t, bh_t, nph_loc),
                    ("pt", wt1_t, bt_t, npt_loc),
                ):
                    pj = pp.tile([128, 128], F32, tag=tag)
                    nc.tensor.matmul(
                        out=pj[:], lhsT=nfT[:], rhs=w1_t[:], start=True, stop=False
                    )
                    nc.tensor.matmul(
                        out=pj[:],
                        lhsT=b1_t[:1, w * 128 : (w + 1) * 128],
                        rhs=b_t[:1, :],
                        start=False,
                        stop=True,
                    )
                    st = pool.tile([128, 128], TDT, tag=tag + "s")
                    nc.any.tensor_copy(out=st[:], in_=pj[:])
                    nc.sync.dma_start(
                        out=loc[w * 128 : (w + 1) * 128, :], in_=st[:]
                    )

            # ---------------- phase B: all-gather projected node tables
            nc.gpsimd.collective_compute(
                "AllGather",
                mybir.AluOpType.bypass,
                replica_groups=rg,
                ins=[nph_loc[:]],
                outs=[nph_gl[:]],
            )
            nc.gpsimd.collective_compute(
                "AllGather",
                mybir.AluOpType.bypass,
                replica_groups=rg,
                ins=[npt_loc[:]],
                outs=[npt_gl[:]],
            )

            # ---------------- phase C: per-edge GEMM + node-row gather
            lo_view_h = nph_gl[: min(R, 32768), :]
            lo_view_t = npt_gl[: min(R, 32768), :]
            for k, (e0, ch) in enumerate(chunks):
                nt = -(-ch // 128)
                ntc = nt * 128
                c0 = e0 // 16
                c1 = (e0 + ch) // 16
                hT = pool.tile([128, CH], DT, tag="hT")
                nc.sync.dma_start(out=hT[:, :ch], in_=efT[0:D, e0 : e0 + ch])
                tT = pool.tile([128, CH], DT, tag="tT")
                nc.sync.dma_start(out=tT[:, :ch], in_=efT[D : 2 * D, e0 : e0 + ch])

                passes = [
                    ("lo_h", lo_view_h, ixlo_h, 0),
                    ("lo_t", lo_view_t, ixlo_t, 1),
                ]
                if two_pass:
                    passes += [
                        ("hi_h", nph_gl[hibase : hibase + 32768, :], ixhi_h, 2),
                        ("hi_t", npt_gl[hibase : hibase + 32768, :], ixhi_t, 3),
                    ]
                gdst = {}
                for key, view, ix_dram, qn in passes:
                    ix_t = pool.tile([128, CH // 16], I16, tag="ix" + key)
                    nc.sync.dma_start(
                        out=ix_t[:, : ch // 16], in_=ix_dram[:, c0:c1]
                    )
                    g = pool.tile([128, CHB * 128], TDT, tag="g" + key)
                    nc.gpsimd.dma_gather(
                        out_ap=g[:, :ntc].rearrange("p (n d) -> p n d", d=D),
                        in_ap=view,
                        idxs_ap=ix_t[:, : ch // 16],
                        num_idxs=ch,
                        num_idxs_reg=ch,
                        elem_size=D,
                        elem_step=D,
                        single_packet=False,
                        queue_num=qn,
                    )
                    gdst[key] = g

                ost = pool.tile([128, CHB * 2 * D], F32, tag="ost")
                for t in range(nt):
                    m = min(128, ch - t * 128)
                    for si, (eT, w2_t, lo_key, hi_key) in enumerate(
                        (
                            (hT, wh2_t, "lo_h", "hi_h"),
                            (tT, wt2_t, "lo_t", "hi_t"),
                        )
                    ):
                        pj = pp.tile([128, 128], F32, tag="pc")
                        nc.tensor.matmul(
                            out=pj[:m, :],
                            lhsT=eT[:, t * 128 : t * 128 + m],
                            rhs=w2_t[:],
                            start=True,
                            stop=True,
                        )
                        o_sl = ost[:m, t * 2 * D + si * D : t * 2 * D + (si + 1) * D]
                        nc.any.tensor_tensor(
                            out=o_sl,
                            in0=pj[:m, :],
                            in1=gdst[lo_key][:m, t * D : (t + 1) * D],
                            op=mybir.AluOpType.add,
                        )
                        if two_pass:
                            nc.any.tensor_tensor(
                                out=o_sl,
                                in0=o_sl,
                                in1=gdst[hi_key][:m, t * D : (t + 1) * D],
                                op=mybir.AluOpType.add,
                            )
                nc.sync.dma_start(
                    out=outw[k * 128 : (k + 1) * 128, : nt * 2 * D],
                    in_=ost[:, : nt * 2 * D],
                )

    nc.compile()
    return nc


# ------------------------------------------------------------------ driver

def _install_ntff_hook():
    """The agent image's antenv lacks axon_hooks; synthesize it so
    run_bass_kernel_spmd(trace=True) can capture NTFF profiles."""
    import types

    try:
        import antenv.axon_hooks  # noqa: F401

        return True
    except ImportError:
        pass
    try:
        import antenv
        from trn_agent_boot.trn_boot import _ntff_profile_via_ctypes

        hook = _ntff_profile_via_ctypes("/opt/axon/libaxon_pjrt.so")
        mod = types.ModuleType("antenv.axon_hooks")
        _state = {"hook": hook}
        mod.set_axon_ntff_profile_hook = lambda h: _state.update(hook=h)
        mod.get_axon_ntff_profile_hook = lambda: _state["hook"]
        sys.modules["antenv.axon_hooks"] = mod
        antenv.axon_hooks = mod
        return hook is not None
    except Exception:
        return False


_CACHE = {}


def _get_program(meta):
    key = (meta["E_c"], meta["W_max"], meta["R"], meta["two_pass"], PREC, CH, T)
    if key not in _CACHE:
        _CACHE[key] = _build(meta)
    return _CACHE[key]


def kernel(
    efeat,
    src,
    dst,
    num_nodes,
    W_head,
    b_head,
    W_tail,
    b_tail,
    _trace=False,
):
    efeat = np.asarray(efeat, dtype=np.float32)
    src = np.asarray(src)
    dst = np.asarray(dst)
    N = int(num_nodes)
    in_maps, meta = _prep(
        efeat,
        src,
        dst,
        N,
        np.asarray(W_head, dtype=np.float32),
        np.asarray(b_head, dtype=np.float32),
        np.asarray(W_tail, dtype=np.float32),
        np.asarray(b_tail, dtype=np.float32),
    )
    nc = _get_program(meta)
    if PREC == "bf16":
        import ml_dtypes

        bf = ml_dtypes.bfloat16
        for m in in_maps:
            for k in ("pay_d", "pay_s", "bias1", "iota", "Wh1", "Wh2",
                      "Wt1", "Wt2", "bh", "bt", "efT"):
                m[k] = m[k].astype(bf)
    if _trace:
        _install_ntff_hook()
    res = run_bass_kernel_spmd(
        nc, in_maps, core_ids=list(range(N_CORES)), trace=_trace
    )
    outs = []
    for c in range(N_CORES):
        ow = res.results[c]["outw"]
        parts = []
        for k, (e0, ch) in enumerate(meta["chunks"]):
            nt = -(-ch // 128)
            blk = (
                ow[k * 128 : (k + 1) * 128, : nt * 2 * D]
                .reshape(128, nt, 2 * D)
                .transpose(1, 0, 2)
                .reshape(nt * 128, 2 * D)
            )
            parts.append(blk[:ch])
        outs.append(np.concatenate(parts, axis=0))
    out = np.concatenate(outs, axis=0)
    if _trace:
        return out, res
    return out


# revision 12
# speedup vs baseline: 1.2568x; 1.0343x over previous
"""Bass/Trainium2 kernel for nn_DirectedLayer (GNN message passing).

Computes, for a directed graph with E edges and N nodes:
    head, tail = split(efeat, 2, axis=-1)
    mean_in  = segment_mean(head, dst, N)
    mean_out = segment_mean(tail, src, N)
    nfeat = 0.5 * (mean_in + mean_out)
    out = concat([concat([nfeat[src], head]) @ W_head + b_head,
                  concat([nfeat[dst], tail]) @ W_tail + b_tail], axis=-1)

Distribution over 8 NeuronCores:
  Phase A: nodes sharded by contiguous windows (<=128 nodes each); each core
           computes the scaled segment sums for its windows via one-hot
           matmuls over host-sorted edge payloads, then projects them through
           the first half of each weight matrix (bias folded in).
  Phase B: AllGather of the two projected node tables.
  Phase C: edges sharded contiguously; each core streams its transposed edge
           features, multiplies by the second half of the weights, gathers the
           projected node rows with 2-pass int16 dma_gather, adds, and stores.
"""

import sys

for p in ("/opt/trn_rl_repo/concourse", "/opt/trn_rl_repo"):
    if p not in sys.path:
        sys.path.insert(0, p)

import numpy as np

import concourse.bass as bass
import concourse.bacc as bacc
import concourse.mybir as mybir
import concourse.tile as tile
from concourse.bass_utils import run_bass_kernel_spmd

N_CORES = 8
D = 128          # feature dim per half
T = 9            # edge tiles (of 128) per window, per direction
CH = 2048        # phase-C chunk size in edges
PREC = "f32r"    # "fp32" | "f32r" | "bf16"
F32 = mybir.dt.float32
I16 = mybir.dt.int16


# ---------------------------------------------------------------- host prep

def _window_partition(cin, cout, N):
    """Greedy split of nodes 0..N-1 into consecutive windows with
    <=128 nodes (127 for window 0; slot 0 is a reserved zero row) and
    <=T*128 incident edges per direction."""
    cap_e = T * 128
    w_id = np.empty(N, dtype=np.int64)
    slot = np.empty(N, dtype=np.int64)
    w = 0
    nn = 0
    de = 0
    se = 0
    first_slot = 1  # window 0 reserves slot 0
    for n in range(N):
        cap_n = 128 - (1 if w == 0 else 0)
        if nn > 0 and (
            nn >= cap_n or de + cin[n] > cap_e or se + cout[n] > cap_e
        ):
            w += 1
            nn = 0
            de = 0
            se = 0
            first_slot = 0
        w_id[n] = w
        slot[n] = nn + (first_slot if w == 0 else 0)
        nn += 1
        de += cin[n]
        se += cout[n]
    return w_id, slot, w + 1


def _dir_payload(efeat_half, nidx, wgt_node, w_id, slot, core, W_max, order):
    """Payload/one-hot metadata for one direction on one core.

    nidx: per-edge node index (dst or src); order: edges sorted by w_id[nidx].
    Returns (pay [W_max*T*128, D], loff [128, W_max*T], wcol [128, W_max*T]).
    """
    w_of_e = w_id[nidx[order]]
    lo_w = core * W_max
    hi_w = (core + 1) * W_max
    m = (w_of_e >= lo_w) & (w_of_e < hi_w)
    ed = order[m]
    we = w_of_e[m] - lo_w
    # rank of each edge within its window (edges are grouped by window)
    starts = np.searchsorted(we, np.arange(W_max))
    rank = np.arange(len(ed)) - starts[we]
    pos = we * (T * 128) + rank
    assert rank.max(initial=0) < T * 128

    rows = W_max * T * 128
    pay = np.zeros((rows, D), dtype=np.float32)
    loff = np.zeros(rows, dtype=np.float32)
    wcol = np.zeros(rows, dtype=np.float32)
    pay[pos] = efeat_half[ed]
    loff[pos] = slot[nidx[ed]]
    wcol[pos] = wgt_node[nidx[ed]]
    pay_w = np.ascontiguousarray(
        pay.reshape(-1, 128, D).transpose(1, 0, 2).reshape(128, -1)
    )
    return (
        pay_w,
        np.ascontiguousarray(loff.reshape(-1, 128).T),
        np.ascontiguousarray(wcol.reshape(-1, 128).T),
    )


def _wrap_idx(gidx, chunks):
    """Pack per-chunk wrapped int16 index table [128, ceil(E_c/16)]."""
    cols = sum(ch for _, ch in chunks) // 16
    out = np.empty((16, cols), dtype=np.int16)
    c0 = 0
    for e0, ch in chunks:
        out[:, c0 : c0 + ch // 16] = (
            gidx[e0 : e0 + ch].reshape(ch // 16, 16).T
        )
        c0 += ch // 16
    return np.ascontiguousarray(np.tile(out, (8, 1)))


def _chunks(E_c):
    out = []
    e0 = 0
    while e0 < E_c:
        ch = min(CH, E_c - e0)
        if ch % 16:
            # keep every chunk a multiple of 16 except never needed for our E
            raise ValueError("edge shard must be a multiple of 16")
        out.append((e0, ch))
        e0 += ch
    return out


def _prep(efeat, src, dst, N, W_head, b_head, W_tail, b_tail):
    E = src.shape[0]
    assert E % N_CORES == 0
    E_c = E // N_CORES
    src = src.astype(np.int64)
    dst = dst.astype(np.int64)

    cin = np.bincount(dst, minlength=N)
    cout = np.bincount(src, minlength=N)
    w_in = (0.5 / np.maximum(cin, 1)).astype(np.float32)
    w_out = (0.5 / np.maximum(cout, 1)).astype(np.float32)

    w_id, slot, W_total = _window_partition(cin, cout, N)
    W_max = -(-W_total // N_CORES)
    # row R-1 must stay a zero row (junk target of the hi gather pass)
    if W_total == N_CORES * W_max:
        last = np.where(w_id == W_total - 1)[0]
        if slot[last].max() >= 127:
            W_max += 1
    R = N_CORES * W_max * 128
    assert R <= 65536, f"node table too large for 2-pass int16 gather: {R}"
    two_pass = R > 32768
    hibase = R - 32768

    row = w_id * 128 + slot  # global row of each node in the gathered tables

    order_d = np.argsort(w_id[dst], kind="stable")
    order_s = np.argsort(w_id[src], kind="stable")

    gh = row[src]  # phase-C gather rows for the head output (nfeat[src])
    gt = row[dst]
    chunks = _chunks(E_c)

    iota = np.tile(np.arange(128, dtype=np.float32), (128, 1))

    in_maps = []
    for c in range(N_CORES):
        pay_d, loff_d, w_d = _dir_payload(
            efeat[:, :D], dst, w_in, w_id, slot, c, W_max, order_d
        )
        pay_s, loff_s, w_s = _dir_payload(
            efeat[:, D:], src, w_out, w_id, slot, c, W_max, order_s
        )
        bias1 = np.zeros((1, W_max * 128), dtype=np.float32)
        mine = (w_id >= c * W_max) & (w_id < (c + 1) * W_max)
        bias1[0, (w_id[mine] - c * W_max) * 128 + slot[mine]] = 1.0

        sl = slice(c * E_c, (c + 1) * E_c)
        ghc = gh[sl]
        gtc = gt[sl]
        if two_pass:
            glo_h = np.where(ghc < 32768, ghc, 0).astype(np.int16)
            ghi_h = np.where(ghc >= 32768, ghc - hibase, 32767).astype(np.int16)
            glo_t = np.where(gtc < 32768, gtc, 0).astype(np.int16)
            ghi_t = np.where(gtc >= 32768, gtc - hibase, 32767).astype(np.int16)
        else:
            glo_h = ghc.astype(np.int16)
            ghi_h = glo_t = ghi_t = None
            glo_t = gtc.astype(np.int16)

        m = {
            "pay_d": pay_d,
            "pay_s": pay_s,
            "loff_d": loff_d,
            "loff_s": loff_s,
            "w_d": w_d,
            "w_s": w_s,
            "bias1": bias1,
            "iota": iota,
            "Wh1": np.ascontiguousarray(W_head[:D]),
            "Wh2": np.ascontiguousarray(W_head[D:]),
            "Wt1": np.ascontiguousarray(W_tail[:D]),
            "Wt2": np.ascontiguousarray(W_tail[D:]),
            "bh": b_head.reshape(1, D).astype(np.float32),
            "bt": b_tail.reshape(1, D).astype(np.float32),
            "efT": np.ascontiguousarray(efeat[sl].T),
            "ixlo_h": _wrap_idx(glo_h, chunks),
            "ixlo_t": _wrap_idx(glo_t, chunks),
        }
        if two_pass:
            m["ixhi_h"] = _wrap_idx(ghi_h, chunks)
            m["ixhi_t"] = _wrap_idx(ghi_t, chunks)
        in_maps.append(m)

    meta = {
        "E_c": E_c,
        "W_max": W_max,
        "R": R,
        "two_pass": two_pass,
        "hibase": hibase,
        "chunks": chunks,
        "idx_cols": sum(ch for _, ch in chunks) // 16,
    }
    return in_maps, meta


# ------------------------------------------------------------- device build

def _build(meta):
    E_c = meta["E_c"]
    W_max = meta["W_max"]
    R = meta["R"]
    two_pass = meta["two_pass"]
    hibase = meta["hibase"]
    chunks = meta["chunks"]
    icols = meta["idx_cols"]
    WT = W_max * T
    CHB = CH // 128

    if PREC == "fp32":
        DT = F32
    elif PREC == "f32r":
        DT = mybir.dt.float32r
    else:
        DT = mybir.dt.bfloat16
    TDT = mybir.dt.bfloat16 if PREC == "bf16" else F32  # node-table dtype

    nc = bacc.Bacc(None, num_devices=N_CORES, debug=False, num_swdge_queues=4)

    pay_d = nc.dram_tensor("pay_d", [128, WT * D], DT, kind="ExternalInput")
    pay_s = nc.dram_tensor("pay_s", [128, WT * D], DT, kind="ExternalInput")
    loff_d = nc.dram_tensor("loff_d", [128, WT], F32, kind="ExternalInput")
    loff_s = nc.dram_tensor("loff_s", [128, WT], F32, kind="ExternalInput")
    w_d = nc.dram_tensor("w_d", [128, WT], F32, kind="ExternalInput")
    w_s = nc.dram_tensor("w_s", [128, WT], F32, kind="ExternalInput")
    bias1 = nc.dram_tensor("bias1", [1, W_max * 128], DT, kind="ExternalInput")
    iota_in = nc.dram_tensor("iota", [128, 128], DT, kind="ExternalInput")
    Wh1 = nc.dram_tensor("Wh1", [D, D], DT, kind="ExternalInput")
    Wh2 = nc.dram_tensor("Wh2", [D, D], DT, kind="ExternalInput")
    Wt1 = nc.dram_tensor("Wt1", [D, D], DT, kind="ExternalInput")
    Wt2 = nc.dram_tensor("Wt2", [D, D], DT, kind="ExternalInput")
    bh = nc.dram_tensor("bh", [1, D], DT, kind="ExternalInput")
    bt = nc.dram_tensor("bt", [1, D], DT, kind="ExternalInput")
    efT = nc.dram_tensor("efT", [2 * D, E_c], DT, kind="ExternalInput")
    ixlo_h = nc.dram_tensor("ixlo_h", [128, icols], I16, kind="ExternalInput")
    ixlo_t = nc.dram_tensor("ixlo_t", [128, icols], I16, kind="ExternalInput")
    if two_pass:
        ixhi_h = nc.dram_tensor("ixhi_h", [128, icols], I16, kind="ExternalInput")
        ixhi_t = nc.dram_tensor("ixhi_t", [128, icols], I16, kind="ExternalInput")
    outw = nc.dram_tensor(
        "outw", [len(chunks) * 128, CHB * 2 * D], F32, kind="ExternalOutput"
    )

    nph_loc = nc.dram_tensor("nph_loc", [W_max * 128, D], TDT)
    npt_loc = nc.dram_tensor("npt_loc", [W_max * 128, D], TDT)
    nph_gl = nc.dram_tensor("nph_gl", [R, D], TDT, addr_space="Shared")
    npt_gl = nc.dram_tensor("npt_gl", [R, D], TDT, addr_space="Shared")

    rg = [list(range(N_CORES))]

    with tile.TileContext(nc) as tc:
        with (
            tc.tile_pool(name="const", bufs=1) as cpool,
            tc.tile_pool(name="sbuf", bufs=2) as pool,
            tc.tile_pool(name="psum", bufs=2, space="PSUM") as pp,
        ):
            iota_t = cpool.tile([128, 128], DT, tag="iota")
            nc.sync.dma_start(out=iota_t[:], in_=iota_in[:])
            wh1_t = cpool.tile([D, D], DT, tag="wh1")
            nc.sync.dma_start(out=wh1_t[:], in_=Wh1[:])
            wh2_t = cpool.tile([D, D], DT, tag="wh2")
            nc.sync.dma_start(out=wh2_t[:], in_=Wh2[:])
            wt1_t = cpool.tile([D, D], DT, tag="wt1")
            nc.sync.dma_start(out=wt1_t[:], in_=Wt1[:])
            wt2_t = cpool.tile([D, D], DT, tag="wt2")
            nc.sync.dma_start(out=wt2_t[:], in_=Wt2[:])
            bh_t = cpool.tile([1, D], DT, tag="bh")
            nc.sync.dma_start(out=bh_t[:], in_=bh[:])
            bt_t = cpool.tile([1, D], DT, tag="bt")
            nc.sync.dma_start(out=bt_t[:], in_=bt[:])
            b1_t = cpool.tile([1, W_max * 128], DT, tag="b1")
            nc.sync.dma_start(out=b1_t[:], in_=bias1[:])
            lo_d_t = cpool.tile([128, WT], F32, tag="lod")
            nc.sync.dma_start(out=lo_d_t[:], in_=loff_d[:])
            lo_s_t = cpool.tile([128, WT], F32, tag="los")
            nc.sync.dma_start(out=lo_s_t[:], in_=loff_s[:])
            wd_t = cpool.tile([128, WT], F32, tag="wd")
            nc.sync.dma_start(out=wd_t[:], in_=w_d[:])
            ws_t = cpool.tile([128, WT], F32, tag="ws")
            nc.sync.dma_start(out=ws_t[:], in_=w_s[:])

            # ---------------- phase A: windowed scaled segment sums + proj
            for w in range(W_max):
                pd = pool.tile([128, T * D], DT, tag="pd")
                nc.sync.dma_start(
                    out=pd[:], in_=pay_d[:, w * T * D : (w + 1) * T * D]
                )
                ps = pool.tile([128, T * D], DT, tag="ps")
                nc.sync.dma_start(
                    out=ps[:], in_=pay_s[:, w * T * D : (w + 1) * T * D]
                )
                psw = pp.tile([128, 128], F32, tag="psw")
                for di, (pay_t, lo_t, wg_t) in enumerate(
                    ((pd, lo_d_t, wd_t), (ps, lo_s_t, ws_t))
                ):
                    for t in range(T):
                        col = w * T + t
                        oh = pool.tile([128, 128], DT, tag="oh")
                        nc.any.tensor_scalar(
                            out=oh[:],
                            in0=iota_t[:],
                            scalar1=lo_t[:, col : col + 1],
                            scalar2=wg_t[:, col : col + 1],
                            op0=mybir.AluOpType.is_equal,
                            op1=mybir.AluOpType.mult,
                        )
                        nc.tensor.matmul(
                            out=psw[:],
                            lhsT=pay_t[:, t * D : (t + 1) * D],
                            rhs=oh[:],
                            start=(di == 0 and t == 0),
                            stop=(di == 1 and t == T - 1),
                        )
                nfT = pool.tile([128, 128], DT, tag="nfT")
                nc.any.tensor_copy(out=nfT[:], in_=psw[:])
                for tag, w1_t, b_t, loc in (
                    ("ph", wh1_t, bh_t, nph_loc),
                    ("pt", wt1_t, bt_t, npt_loc),
                ):
                    pj = pp.tile([128, 128], F32, tag=tag)
                    nc.tensor.matmul(
                        out=pj[:], lhsT=nfT[:], rhs=w1_t[:], start=True, stop=False
                    )
                    nc.tensor.matmul(
                        out=pj[:],
                        lhsT=b1_t[:1, w * 128 : (w + 1) * 128],
                        rhs=b_t[:1, :],
                        start=False,
                        stop=True,
                    )
                    st = pool.tile([128, 128], TDT, tag=tag + "s")
                    nc.any.tensor_copy(out=st[:], in_=pj[:])
                    nc.sync.dma_start(
                        out=loc[w * 128 : (w + 1) * 128, :], in_=st[:]
                    )

            # ---------------- phase B: all-gather projected node tables
            nc.gpsimd.collective_compute(
                "AllGather",
                mybir.AluOpType.bypass,
                replica_groups=rg,
                ins=[nph_loc[:]],
                outs=[nph_gl[:]],
            )
            nc.gpsimd.collective_compute(
                "AllGather",
                mybir.AluOpType.bypass,
                replica_groups=rg,
                ins=[npt_loc[:]],
                outs=[npt_gl[:]],
            )

            # ---------------- phase C: per-edge GEMM + node-row gather
            lo_view_h = nph_gl[: min(R, 32768), :]
            lo_view_t = npt_gl[: min(R, 32768), :]
            for k, (e0, ch) in enumerate(chunks):
                nt = -(-ch // 128)
                ntc = nt * 128
                c0 = e0 // 16
                c1 = (e0 + ch) // 16
                hT = pool.tile([128, CH], DT, tag="hT")
                nc.sync.dma_start(out=hT[:, :ch], in_=efT[0:D, e0 : e0 + ch])
                tT = pool.tile([128, CH], DT, tag="tT")
                nc.sync.dma_start(out=tT[:, :ch], in_=efT[D : 2 * D, e0 : e0 + ch])

                passes = [
                    ("lo_h", lo_view_h, ixlo_h, 0),
                    ("lo_t", lo_view_t, ixlo_t, 1),
                ]
                if two_pass:
                    passes += [
                        ("hi_h", nph_gl[hibase : hibase + 32768, :], ixhi_h, 2),
                        ("hi_t", npt_gl[hibase : hibase + 32768, :], ixhi_t, 3),
                    ]
                gdst = {}
                for key, view, ix_dram, qn in passes:
                    ix_t = pool.tile([128, CH // 16], I16, tag="ix" + key)
                    nc.sync.dma_start(
                        out=ix_t[:, : ch // 16], in_=ix_dram[:, c0:c1]
                    )
                    g = pool.tile([128, CHB * 128], TDT, tag="g" + key)
                    nc.gpsimd.dma_gather(
                        out_ap=g[:, :ntc].rearrange("p (n d) -> p n d", d=D),
                        in_ap=view,
                        idxs_ap=ix_t[:, : ch // 16],
                        num_idxs=ch,
                        num_idxs_reg=ch,
                        elem_size=D,
                        elem_step=D,
                        single_packet=False,
                        queue_num=qn,
                    )
                    gdst[key] = g

                ost = pool.tile([128, CHB * 2 * D], F32, tag="ost")
                for t in range(nt):
                    m = min(128, ch - t * 128)
                    for si, (eT, w2_t, lo_key, hi_key) in enumerate(
                        (
                            (hT, wh2_t, "lo_h", "hi_h"),
                            (tT, wt2_t, "lo_t", "hi_t"),
                        )
                    ):
                        pj = pp.tile([128, 128], F32, tag="pc")
                        nc.tensor.matmul(
                            out=pj[:m, :],
                            lhsT=eT[:, t * 128 : t * 128 + m],
                            rhs=w2_t[:],
                            start=True,
                            stop=True,
                        )
                        o_sl = ost[:m, t * 2 * D + si * D : t * 2 * D + (si + 1) * D]
                        # copy PSUM out immediately so the PE can run ahead;
                        # the gather-dependent adds land in SBUF later
                        nc.any.tensor_copy(out=o_sl, in_=pj[:m, :])
                        nc.any.tensor_tensor(
                            out=o_sl,
                            in0=o_sl,
                            in1=gdst[lo_key][:m, t * D : (t + 1) * D],
                            op=mybir.AluOpType.add,
                        )
                        if two_pass:
                            nc.any.tensor_tensor(
                                out=o_sl,
                                in0=o_sl,
                                in1=gdst[hi_key][:m, t * D : (t + 1) * D],
                                op=mybir.AluOpType.add,
                            )
                nc.sync.dma_start(
                    out=outw[k * 128 : (k + 1) * 128, : nt * 2 * D],
                    in_=ost[:, : nt * 2 * D],
                )

    nc.compile()
    return nc


# ------------------------------------------------------------------ driver

def _install_ntff_hook():
    """The agent image's antenv lacks axon_hooks; synthesize it so
    run_bass_kernel_spmd(trace=True) can capture NTFF profiles."""
    import types

    try:
        import antenv.axon_hooks  # noqa: F401

        return True
    except ImportError:
        pass
    try:
        import antenv
        from trn_agent_boot.trn_boot import _ntff_profile_via_ctypes

        hook = _ntff_profile_via_ctypes("/opt/axon/libaxon_pjrt.so")
        mod = types.ModuleType("antenv.axon_hooks")
        _state = {"hook": hook}
        mod.set_axon_ntff_profile_hook = lambda h: _state.update(hook=h)
        mod.get_axon_ntff_profile_hook = lambda: _state["hook"]
        sys.modules["antenv.axon_hooks"] = mod
        antenv.axon_hooks = mod
        return hook is not None
    except Exception:
        return False


_CACHE = {}


def _get_program(meta):
    key = (meta["E_c"], meta["W_max"], meta["R"], meta["two_pass"], PREC, CH, T)
    if key not in _CACHE:
        _CACHE[key] = _build(meta)
    return _CACHE[key]


def kernel(
    efeat,
    src,
    dst,
    num_nodes,
    W_head,
    b_head,
    W_tail,
    b_tail,
    _trace=False,
):
    efeat = np.asarray(efeat, dtype=np.float32)
    src = np.asarray(src)
    dst = np.asarray(dst)
    N = int(num_nodes)
    in_maps, meta = _prep(
        efeat,
        src,
        dst,
        N,
        np.asarray(W_head, dtype=np.float32),
        np.asarray(b_head, dtype=np.float32),
        np.asarray(W_tail, dtype=np.float32),
        np.asarray(b_tail, dtype=np.float32),
    )
    nc = _get_program(meta)
    if PREC == "bf16":
        import ml_dtypes

        bf = ml_dtypes.bfloat16
        for m in in_maps:
            for k in ("pay_d", "pay_s", "bias1", "iota", "Wh1", "Wh2",
                      "Wt1", "Wt2", "bh", "bt", "efT"):
                m[k] = m[k].astype(bf)
    if _trace:
        _install_ntff_hook()
    res = run_bass_kernel_spmd(
        nc, in_maps, core_ids=list(range(N_CORES)), trace=_trace
    )
    outs = []
    for c in range(N_CORES):
        ow = res.results[c]["outw"]
        parts = []
        for k, (e0, ch) in enumerate(meta["chunks"]):
            nt = -(-ch // 128)
            blk = (
                ow[k * 128 : (k + 1) * 128, : nt * 2 * D]
                .reshape(128, nt, 2 * D)
                .transpose(1, 0, 2)
                .reshape(nt * 128, 2 * D)
            )
            parts.append(blk[:ch])
        outs.append(np.concatenate(parts, axis=0))
    out = np.concatenate(outs, axis=0)
    if _trace:
        return out, res
    return out


# revision 14
# speedup vs baseline: 1.2909x; 1.0272x over previous
"""Bass/Trainium2 kernel for nn_DirectedLayer (GNN message passing).

Computes, for a directed graph with E edges and N nodes:
    head, tail = split(efeat, 2, axis=-1)
    mean_in  = segment_mean(head, dst, N)
    mean_out = segment_mean(tail, src, N)
    nfeat = 0.5 * (mean_in + mean_out)
    out = concat([concat([nfeat[src], head]) @ W_head + b_head,
                  concat([nfeat[dst], tail]) @ W_tail + b_tail], axis=-1)

Distribution over 8 NeuronCores:
  Phase A: nodes sharded by contiguous windows (<=128 nodes each); each core
           computes the scaled segment sums for its windows via one-hot
           matmuls over host-sorted edge payloads, then projects them through
           the first half of each weight matrix (bias folded in).
  Phase B: AllGather of the two projected node tables.
  Phase C: edges sharded contiguously; each core streams its transposed edge
           features, multiplies by the second half of the weights, gathers the
           projected node rows with 2-pass int16 dma_gather, adds, and stores.
"""

import sys

for p in ("/opt/trn_rl_repo/concourse", "/opt/trn_rl_repo"):
    if p not in sys.path:
        sys.path.insert(0, p)

import numpy as np

import concourse.bass as bass
import concourse.bacc as bacc
import concourse.mybir as mybir
import concourse.tile as tile
from concourse.bass_utils import run_bass_kernel_spmd

N_CORES = 8
D = 128          # feature dim per half
T = 9            # edge tiles (of 128) per window, per direction
CH = 2048        # phase-C chunk size in edges
PREC = "f32r"    # "fp32" | "f32r" | "bf16"
F32 = mybir.dt.float32
I16 = mybir.dt.int16


# ---------------------------------------------------------------- host prep

def _window_partition(cin, cout, N):
    """Greedy split of nodes 0..N-1 into consecutive windows with
    <=128 nodes (127 for window 0; slot 0 is a reserved zero row) and
    <=T*128 incident edges per direction."""
    cap_e = T * 128
    w_id = np.empty(N, dtype=np.int64)
    slot = np.empty(N, dtype=np.int64)
    w = 0
    nn = 0
    de = 0
    se = 0
    first_slot = 1  # window 0 reserves slot 0
    for n in range(N):
        cap_n = 128 - (1 if w == 0 else 0)
        if nn > 0 and (
            nn >= cap_n or de + cin[n] > cap_e or se + cout[n] > cap_e
        ):
            w += 1
            nn = 0
            de = 0
            se = 0
            first_slot = 0
        w_id[n] = w
        slot[n] = nn + (first_slot if w == 0 else 0)
        nn += 1
        de += cin[n]
        se += cout[n]
    return w_id, slot, w + 1


def _dir_payload(efeat_half, nidx, wgt_node, w_id, slot, core, W_max, order):
    """Payload/one-hot metadata for one direction on one core.

    nidx: per-edge node index (dst or src); order: edges sorted by w_id[nidx].
    Returns (pay [W_max*T*128, D], loff [128, W_max*T], wcol [128, W_max*T]).
    """
    w_of_e = w_id[nidx[order]]
    lo_w = core * W_max
    hi_w = (core + 1) * W_max
    m = (w_of_e >= lo_w) & (w_of_e < hi_w)
    ed = order[m]
    we = w_of_e[m] - lo_w
    # rank of each edge within its window (edges are grouped by window)
    starts = np.searchsorted(we, np.arange(W_max))
    rank = np.arange(len(ed)) - starts[we]
    pos = we * (T * 128) + rank
    assert rank.max(initial=0) < T * 128

    rows = W_max * T * 128
    pay = np.zeros((rows, D), dtype=np.float32)
    loff = np.zeros(rows, dtype=np.float32)
    wcol = np.zeros(rows, dtype=np.float32)
    pay[pos] = efeat_half[ed]
    loff[pos] = slot[nidx[ed]]
    wcol[pos] = wgt_node[nidx[ed]]
    pay_w = np.ascontiguousarray(
        pay.reshape(-1, 128, D).transpose(1, 0, 2).reshape(128, -1)
    )
    return (
        pay_w,
        np.ascontiguousarray(loff.reshape(-1, 128).T),
        np.ascontiguousarray(wcol.reshape(-1, 128).T),
    )


def _wrap_idx(gidx, chunks):
    """Pack per-chunk wrapped int16 index table [128, ceil(E_c/16)]."""
    cols = sum(ch for _, ch in chunks) // 16
    out = np.empty((16, cols), dtype=np.int16)
    c0 = 0
    for e0, ch in chunks:
        out[:, c0 : c0 + ch // 16] = (
            gidx[e0 : e0 + ch].reshape(ch // 16, 16).T
        )
        c0 += ch // 16
    return np.ascontiguousarray(np.tile(out, (8, 1)))


def _chunks(E_c):
    out = []
    e0 = 0
    while e0 < E_c:
        ch = min(CH, E_c - e0)
        if ch % 16:
            # keep every chunk a multiple of 16 except never needed for our E
            raise ValueError("edge shard must be a multiple of 16")
        out.append((e0, ch))
        e0 += ch
    return out


def _prep(efeat, src, dst, N, W_head, b_head, W_tail, b_tail):
    E = src.shape[0]
    assert E % N_CORES == 0
    E_c = E // N_CORES
    src = src.astype(np.int64)
    dst = dst.astype(np.int64)

    cin = np.bincount(dst, minlength=N)
    cout = np.bincount(src, minlength=N)
    w_in = (0.5 / np.maximum(cin, 1)).astype(np.float32)
    w_out = (0.5 / np.maximum(cout, 1)).astype(np.float32)

    w_id, slot, W_total = _window_partition(cin, cout, N)
    W_max = -(-W_total // N_CORES)
    if W_total == N_CORES * W_max:
        last = np.where(w_id == W_total - 1)[0]
        if slot[last].max() >= 127:
            W_max += 1
    # two slabs of S windows; each slab's gathered table is <=32768 rows
    K = 1 if W_max <= 32 else 2
    S = -(-W_max // K)
    W_max = K * S
    assert S <= 32, f"slab too large for int16 gather: S={S}"
    RS = N_CORES * S * 128  # rows per gathered slab

    core_of = w_id // W_max
    wl = w_id - core_of * W_max         # window local to core
    slab = wl // S
    # row within the slab's gathered table
    srow = core_of * S * 128 + (wl - slab * S) * 128 + slot
    # slab-1 junk target is its last row: must be an unassigned slot
    if K == 2:
        bad = (slab == 1) & (srow == RS - 1)
        assert not bad.any(), "slab-1 junk row is occupied; adjust W_max"

    order_d = np.argsort(w_id[dst], kind="stable")
    order_s = np.argsort(w_id[src], kind="stable")

    chunks = _chunks(E_c)

    iota = np.tile(np.arange(128, dtype=np.float32), (128, 1))

    in_maps = []
    for c in range(N_CORES):
        pay_d, loff_d, w_d = _dir_payload(
            efeat[:, :D], dst, w_in, w_id, slot, c, W_max, order_d
        )
        pay_s, loff_s, w_s = _dir_payload(
            efeat[:, D:], src, w_out, w_id, slot, c, W_max, order_s
        )
        bias1 = np.zeros((1, W_max * 128), dtype=np.float32)
        mine = (w_id >= c * W_max) & (w_id < (c + 1) * W_max)
        bias1[0, (w_id[mine] - c * W_max) * 128 + slot[mine]] = 1.0

        sl = slice(c * E_c, (c + 1) * E_c)
        m = {
            "pay_d": pay_d,
            "pay_s": pay_s,
            "loff_d": loff_d,
            "loff_s": loff_s,
            "w_d": w_d,
            "w_s": w_s,
            "bias1": bias1,
            "iota": iota,
            "Wh1": np.ascontiguousarray(W_head[:D]),
            "Wh2": np.ascontiguousarray(W_head[D:]),
            "Wt1": np.ascontiguousarray(W_tail[:D]),
            "Wt2": np.ascontiguousarray(W_tail[D:]),
            "bh": b_head.reshape(1, D).astype(np.float32),
            "bt": b_tail.reshape(1, D).astype(np.float32),
            "efT": np.ascontiguousarray(efeat[sl].T),
        }
        for key, nidx in (("h", src[sl]), ("t", dst[sl])):
            sb = slab[nidx]
            rw = srow[nidx]
            if K == 2:
                lo = np.where(sb == 0, rw, 0).astype(np.int16)
                hi = np.where(sb == 1, rw, RS - 1).astype(np.int16)
                m["ixhi_" + key] = _wrap_idx(hi, chunks)
            else:
                lo = rw.astype(np.int16)
            m["ixlo_" + key] = _wrap_idx(lo, chunks)
        in_maps.append(m)

    meta = {
        "E_c": E_c,
        "W_max": W_max,
        "S": S,
        "K": K,
        "RS": RS,
        "chunks": chunks,
        "idx_cols": sum(ch for _, ch in chunks) // 16,
    }
    return in_maps, meta


# ------------------------------------------------------------- device build

def _build(meta):
    E_c = meta["E_c"]
    W_max = meta["W_max"]
    S = meta["S"]
    K = meta["K"]
    RS = meta["RS"]
    chunks = meta["chunks"]
    icols = meta["idx_cols"]
    WT = W_max * T
    CHB = CH // 128

    if PREC == "fp32":
        DT = F32
    elif PREC == "f32r":
        DT = mybir.dt.float32r
    else:
        DT = mybir.dt.bfloat16
    TDT = mybir.dt.bfloat16 if PREC == "bf16" else F32  # node-table dtype

    nc = bacc.Bacc(None, num_devices=N_CORES, debug=False, num_swdge_queues=4)

    pay_d = nc.dram_tensor("pay_d", [128, WT * D], DT, kind="ExternalInput")
    pay_s = nc.dram_tensor("pay_s", [128, WT * D], DT, kind="ExternalInput")
    loff_d = nc.dram_tensor("loff_d", [128, WT], F32, kind="ExternalInput")
    loff_s = nc.dram_tensor("loff_s", [128, WT], F32, kind="ExternalInput")
    w_d = nc.dram_tensor("w_d", [128, WT], F32, kind="ExternalInput")
    w_s = nc.dram_tensor("w_s", [128, WT], F32, kind="ExternalInput")
    bias1 = nc.dram_tensor("bias1", [1, W_max * 128], DT, kind="ExternalInput")
    iota_in = nc.dram_tensor("iota", [128, 128], DT, kind="ExternalInput")
    Wh1 = nc.dram_tensor("Wh1", [D, D], DT, kind="ExternalInput")
    Wh2 = nc.dram_tensor("Wh2", [D, D], DT, kind="ExternalInput")
    Wt1 = nc.dram_tensor("Wt1", [D, D], DT, kind="ExternalInput")
    Wt2 = nc.dram_tensor("Wt2", [D, D], DT, kind="ExternalInput")
    bh = nc.dram_tensor("bh", [1, D], DT, kind="ExternalInput")
    bt = nc.dram_tensor("bt", [1, D], DT, kind="ExternalInput")
    efT = nc.dram_tensor("efT", [2 * D, E_c], DT, kind="ExternalInput")
    ix_in = {}
    for key in ("h", "t"):
        ix_in["lo_" + key] = nc.dram_tensor(
            "ixlo_" + key, [128, icols], I16, kind="ExternalInput"
        )
        if K == 2:
            ix_in["hi_" + key] = nc.dram_tensor(
                "ixhi_" + key, [128, icols], I16, kind="ExternalInput"
            )
    outw = nc.dram_tensor(
        "outw", [len(chunks) * 128, CHB * 2 * D], F32, kind="ExternalOutput"
    )

    loc = {}
    gl = {}
    for key in ("h", "t"):
        for j in range(K):
            loc[key, j] = nc.dram_tensor(f"np{key}_loc{j}", [S * 128, D], TDT)
            gl[key, j] = nc.dram_tensor(
                f"np{key}_gl{j}", [RS, D], TDT, addr_space="Shared"
            )

    rg = [list(range(N_CORES))]

    with tile.TileContext(nc) as tc:
        with (
            tc.tile_pool(name="const", bufs=1) as cpool,
            tc.tile_pool(name="sbuf", bufs=2) as pool,
            tc.tile_pool(name="psum", bufs=2, space="PSUM") as pp,
        ):
            iota_t = cpool.tile([128, 128], DT, tag="iota")
            nc.sync.dma_start(out=iota_t[:], in_=iota_in[:])
            wh1_t = cpool.tile([D, D], DT, tag="wh1")
            nc.sync.dma_start(out=wh1_t[:], in_=Wh1[:])
            wh2_t = cpool.tile([D, D], DT, tag="wh2")
            nc.sync.dma_start(out=wh2_t[:], in_=Wh2[:])
            wt1_t = cpool.tile([D, D], DT, tag="wt1")
            nc.sync.dma_start(out=wt1_t[:], in_=Wt1[:])
            wt2_t = cpool.tile([D, D], DT, tag="wt2")
            nc.sync.dma_start(out=wt2_t[:], in_=Wt2[:])
            bh_t = cpool.tile([1, D], DT, tag="bh")
            nc.sync.dma_start(out=bh_t[:], in_=bh[:])
            bt_t = cpool.tile([1, D], DT, tag="bt")
            nc.sync.dma_start(out=bt_t[:], in_=bt[:])
            b1_t = cpool.tile([1, W_max * 128], DT, tag="b1")
            nc.sync.dma_start(out=b1_t[:], in_=bias1[:])
            lo_d_t = cpool.tile([128, WT], F32, tag="lod")
            nc.sync.dma_start(out=lo_d_t[:], in_=loff_d[:])
            lo_s_t = cpool.tile([128, WT], F32, tag="los")
            nc.sync.dma_start(out=lo_s_t[:], in_=loff_s[:])
            wd_t = cpool.tile([128, WT], F32, tag="wd")
            nc.sync.dma_start(out=wd_t[:], in_=w_d[:])
            ws_t = cpool.tile([128, WT], F32, tag="ws")
            nc.sync.dma_start(out=ws_t[:], in_=w_s[:])

            # ---------------- phase A: windowed scaled segment sums + proj
            # per-slab; the slab's AllGather fires as soon as it completes
            for j in range(K):
                for wloc in range(S):
                    w = j * S + wloc
                    pd = pool.tile([128, T * D], DT, tag="pd")
                    nc.sync.dma_start(
                        out=pd[:], in_=pay_d[:, w * T * D : (w + 1) * T * D]
                    )
                    ps = pool.tile([128, T * D], DT, tag="ps")
                    nc.sync.dma_start(
                        out=ps[:], in_=pay_s[:, w * T * D : (w + 1) * T * D]
                    )
                    psw = pp.tile([128, 128], F32, tag="psw")
                    for di, (pay_t, lo_t, wg_t) in enumerate(
                        ((pd, lo_d_t, wd_t), (ps, lo_s_t, ws_t))
                    ):
                        for t in range(T):
                            col = w * T + t
                            oh = pool.tile([128, 128], DT, tag="oh")
                            nc.any.tensor_scalar(
                                out=oh[:],
                                in0=iota_t[:],
                                scalar1=lo_t[:, col : col + 1],
                                scalar2=wg_t[:, col : col + 1],
                                op0=mybir.AluOpType.is_equal,
                                op1=mybir.AluOpType.mult,
                            )
                            nc.tensor.matmul(
                                out=psw[:],
                                lhsT=pay_t[:, t * D : (t + 1) * D],
                                rhs=oh[:],
                                start=(di == 0 and t == 0),
                                stop=(di == 1 and t == T - 1),
                            )
                    nfT = pool.tile([128, 128], DT, tag="nfT")
                    nc.any.tensor_copy(out=nfT[:], in_=psw[:])
                    for tag, w1_t, b_t, key in (
                        ("ph", wh1_t, bh_t, "h"),
                        ("pt", wt1_t, bt_t, "t"),
                    ):
                        pj = pp.tile([128, 128], F32, tag=tag)
                        nc.tensor.matmul(
                            out=pj[:],
                            lhsT=nfT[:],
                            rhs=w1_t[:],
                            start=True,
                            stop=False,
                        )
                        nc.tensor.matmul(
                            out=pj[:],
                            lhsT=b1_t[:1, w * 128 : (w + 1) * 128],
                            rhs=b_t[:1, :],
                            start=False,
                            stop=True,
                        )
                        st = pool.tile([128, 128], TDT, tag=tag + "s")
                        nc.any.tensor_copy(out=st[:], in_=pj[:])
                        nc.sync.dma_start(
                            out=loc[key, j][wloc * 128 : (wloc + 1) * 128, :],
                            in_=st[:],
                        )
                # ---- phase B (slab j): all-gather this slab's tables
                for key in ("h", "t"):
                    nc.gpsimd.collective_compute(
                        "AllGather",
                        mybir.AluOpType.bypass,
                        replica_groups=rg,
                        ins=[loc[key, j][:]],
                        outs=[gl[key, j][:]],
                    )

            # ---------------- phase C: per-edge GEMM + node-row gather
            for k, (e0, ch) in enumerate(chunks):
                nt = -(-ch // 128)
                ntc = nt * 128
                c0 = e0 // 16
                c1 = (e0 + ch) // 16
                hT = pool.tile([128, CH], DT, tag="hT")
                nc.sync.dma_start(out=hT[:, :ch], in_=efT[0:D, e0 : e0 + ch])
                tT = pool.tile([128, CH], DT, tag="tT")
                nc.sync.dma_start(out=tT[:, :ch], in_=efT[D : 2 * D, e0 : e0 + ch])

                passes = [("lo_h", 0), ("lo_t", 1)]
                if K == 2:
                    passes += [("hi_h", 2), ("hi_t", 3)]
                gdst = {}
                for key, qn in passes:
                    j = 0 if key.startswith("lo") else 1
                    tk = key[-1]
                    ix_t = pool.tile([128, CH // 16], I16, tag="ix" + key)
                    nc.sync.dma_start(
                        out=ix_t[:, : ch // 16], in_=ix_in[key][:, c0:c1]
                    )
                    g = pool.tile([128, CHB * 128], TDT, tag="g" + key)
                    nc.gpsimd.dma_gather(
                        out_ap=g[:, :ntc].rearrange("p (n d) -> p n d", d=D),
                        in_ap=gl[tk, j][:],
                        idxs_ap=ix_t[:, : ch // 16],
                        num_idxs=ch,
                        num_idxs_reg=ch,
                        elem_size=D,
                        elem_step=D,
                        single_packet=False,
                        queue_num=qn,
                    )
                    gdst[key] = g

                ost = pool.tile([128, CHB * 2 * D], F32, tag="ost")
                for t in range(nt):
                    m = min(128, ch - t * 128)
                    for si, (eT, w2_t, lo_key, hi_key) in enumerate(
                        (
                            (hT, wh2_t, "lo_h", "hi_h"),
                            (tT, wt2_t, "lo_t", "hi_t"),
                        )
                    ):
                        pj = pp.tile([128, 128], F32, tag="pc")
                        nc.tensor.matmul(
                            out=pj[:m, :],
                            lhsT=eT[:, t * 128 : t * 128 + m],
                            rhs=w2_t[:],
                            start=True,
                            stop=True,
                        )
                        o_sl = ost[:m, t * 2 * D + si * D : t * 2 * D + (si + 1) * D]
                        nc.any.tensor_tensor(
                            out=o_sl,
                            in0=pj[:m, :],
                            in1=gdst[lo_key][:m, t * D : (t + 1) * D],
                            op=mybir.AluOpType.add,
                        )
                        if K == 2:
                            nc.any.tensor_tensor(
                                out=o_sl,
                                in0=o_sl,
                                in1=gdst[hi_key][:m, t * D : (t + 1) * D],
                                op=mybir.AluOpType.add,
                            )
                nc.sync.dma_start(
                    out=outw[k * 128 : (k + 1) * 128, : nt * 2 * D],
                    in_=ost[:, : nt * 2 * D],
                )

    nc.compile()
    return nc


# ------------------------------------------------------------------ driver

def _install_ntff_hook():
    """The agent image's antenv lacks axon_hooks; synthesize it so
    run_bass_kernel_spmd(trace=True) can capture NTFF profiles."""
    import types

    try:
        import antenv.axon_hooks  # noqa: F401

        return True
    except ImportError:
        pass
    try:
        import antenv
        from trn_agent_boot.trn_boot import _ntff_profile_via_ctypes

        hook = _ntff_profile_via_ctypes("/opt/axon/libaxon_pjrt.so")
        mod = types.ModuleType("antenv.axon_hooks")
        _state = {"hook": hook}
        mod.set_axon_ntff_profile_hook = lambda h: _state.update(hook=h)
        mod.get_axon_ntff_profile_hook = lambda: _state["hook"]
        sys.modules["antenv.axon_hooks"] = mod
        antenv.axon_hooks = mod
        return hook is not None
    except Exception:
        return False


_CACHE = {}


def _get_program(meta):
    key = (meta["E_c"], meta["W_max"], meta["S"], meta["K"], PREC, CH, T)
    if key not in _CACHE:
        _CACHE[key] = _build(meta)
    return _CACHE[key]


def kernel(
    efeat,
    src,
    dst,
    num_nodes,
    W_head,
    b_head,
    W_tail,
    b_tail,
    _trace=False,
):
    efeat = np.asarray(efeat, dtype=np.float32)
    src = np.asarray(src)
    dst = np.asarray(dst)
    N = int(num_nodes)
    in_maps, meta = _prep(
        efeat,
        src,
        dst,
        N,
        np.asarray(W_head, dtype=np.float32),
        np.asarray(b_head, dtype=np.float32),
        np.asarray(W_tail, dtype=np.float32),
        np.asarray(b_tail, dtype=np.float32),
    )
    nc = _get_program(meta)
    if PREC == "bf16":
        import ml_dtypes

        bf = ml_dtypes.bfloat16
        for m in in_maps:
            for k in ("pay_d", "pay_s", "bias1", "iota", "Wh1", "Wh2",
                      "Wt1", "Wt2", "bh", "bt", "efT"):
                m[k] = m[k].astype(bf)
    if _trace:
        _install_ntff_hook()
    res = run_bass_kernel_spmd(
        nc, in_maps, core_ids=list(range(N_CORES)), trace=_trace
    )
    outs = []
    for c in range(N_CORES):
        ow = res.results[c]["outw"]
        parts = []
        for k, (e0, ch) in enumerate(meta["chunks"]):
            nt = -(-ch // 128)
            blk = (
                ow[k * 128 : (k + 1) * 128, : nt * 2 * D]
                .reshape(128, nt, 2 * D)
                .transpose(1, 0, 2)
                .reshape(nt * 128, 2 * D)
            )
            parts.append(blk[:ch])
        outs.append(np.concatenate(parts, axis=0))
    out = np.concatenate(outs, axis=0)
    if _trace:
        return out, res
    return out
